# revision 1
# baseline (speedup 1.0000x reference)
"""Bass/TRN2 kernel for nn_BiRNNLayers: 2-layer BiLSTM (B=64, T=512, H=128,
vocab 50000) with masked Keras-style scan, feature pooling and FC head.

Strategy (8 NeuronCores, data-parallel over batch, 8 rows/core):
- Embedding gather on device (indirect DMA, one row per partition).
- Single-activation-table trick: all 4 gates computed with one tanh over the
  [128, 4, B] gate block (sigmoid = (1+tanh(z/2))/2 folded into weights);
  state kept as H'=2h, C=2c so no per-step scaling ops are needed.
- Transposed (H-on-partitions) layout: the recurrent matmul consumes H'
  directly as the moving operand, no per-step transposes.
- Masked carry: c-carry is exact via gate saturation (+-20 pre-tanh folded
  into xp at precompute time), h-carry via copy_predicated with a u8 mask.
- Layer outputs stream to DRAM; layer-1 xp, pooling and FC read them back
  with (possibly time-reversed) strided access patterns.
"""
import numpy as np

import concourse.bass as bass
import concourse.mybir as mybir
import concourse.tile as tile
import bass_rust

P = 128
T = 512
H = 128
E = 128
B_FULL = 64
NCORES = 8
BC = B_FULL // NCORES  # batch rows per core
VOCAB = 50000
NCLS = 10
KSAT = 40.0            # pre-activation saturation offset for masked steps
UNROLL = 1

AF = mybir.ActivationFunctionType
ALU = mybir.AluOpType
dt = mybir.dt

_hook_installed = False


def _install_hook():
    """Surface compile-hook tracebacks (PJRT swallows them otherwise)."""
    global _hook_installed
    if _hook_installed:
        return
    _hook_installed = True
    import traceback
    import concourse.bass2jax as bass2jax
    import libneuronxla

    orig = bass2jax.neuronx_cc_hook

    def dbg_hook(*a, **k):
        try:
            return orig(*a, **k)
        except BaseException:
            traceback.print_exc()
            raise

    bass2jax.neuronx_cc_hook = dbg_hook
    if not hasattr(libneuronxla, "orig_neuronx_cc"):
        libneuronxla.orig_neuronx_cc = libneuronxla.neuronx_cc
    libneuronxla.neuronx_cc = dbg_hook


def split_multi_waits(nc):
    """This container's walrus encodes at most one sem wait per instruction;
    hoist extra waits onto preceding same-engine NoOps."""
    for fn in nc.m.functions:
        for bb in fn.blocks:
            out = []
            changed = False
            for inst in bb.instructions:
                si = inst.sync_info
                waits = list(si.on_wait) if si is not None and si.on_wait else []
                if len(waits) > 1:
                    changed = True
                    for k, w in enumerate(waits[:-1]):
                        nop = mybir.InstNoOp(name=f"{inst.name}-sw{k}")
                        nop.engine = inst.engine
                        nop.sync_info = bass_rust.SyncInfo(on_wait=[w], on_update=[])
                        out.append(nop)
                    inst.sync_info = bass_rust.SyncInfo(
                        on_wait=[waits[-1]], on_update=list(si.on_update)
                    )
                out.append(inst)
            if changed:
                bb.instructions = out


# ---------------------------------------------------------------------------
# host-side weight folding
# ---------------------------------------------------------------------------

def _fold_weights(inputs):
    f32 = np.float32
    # gate column scaling: sigmoid gates (i, f, o) evaluated as tanh(z/2)
    cs = np.concatenate([
        np.full(H, 0.5), np.full(H, 0.5), np.ones(H), np.full(H, 0.5)
    ]).astype(f32)

    w = {}
    for l in (0, 1):
        for d in ("f", "b"):
            Wx = np.asarray(inputs[f"Wx_{d}{l}"], f32)
            Wh = np.asarray(inputs[f"Wh_{d}{l}"], f32)
            b = np.asarray(inputs[f"b_{d}{l}"], f32)
            w[f"wh{l}{d}"] = ((Wh * 0.5) * cs).astype(f32)
            be = (b * cs).astype(f32)
            w[f"bcol{l}{d}"] = np.ascontiguousarray(
                be.reshape(4, H).T)  # [128, 4]
            if l == 0:
                w[f"wx0{d}a"] = (Wx * cs).astype(f32)
            else:
                # rows 0:128 multiply y0f' = 2*hf, rows 128:256 multiply y0b'
                w[f"wx1{d}f"] = ((Wx[0:H] * 0.5) * cs).astype(f32)
                w[f"wx1{d}b"] = ((Wx[H:2 * H] * 0.5) * cs).astype(f32)

    w["emb"] = np.asarray(inputs["emb"], f32)

    fcw = np.asarray(inputs["fc_W"], f32).copy()  # [2T, 10]
    fcw[:T] *= 0.5          # mx rows: feat carries 2*mx
    fcw[T:] *= 1.0 / 512.0  # av rows: feat carries sum(2h) over 256 feats
    w["fcw"] = fcw.astype(f32)
    w["fcb_rep"] = np.tile(np.asarray(inputs["fc_b"], f32)[None, :], (BC, 1))
    w["ident"] = np.eye(P, dtype=f32)
    return w


# ---------------------------------------------------------------------------
# device program
# ---------------------------------------------------------------------------

def _build():
    nc = bass.Bass("TRN2", target_bir_lowering=False, debug=False,
                   num_devices=NCORES)

    def di(name, shape, dtype=dt.float32):
        return nc.dram_tensor(name, shape, dtype, kind="ExternalInput")

    emb_d = di("emb", [VOCAB + 1, E])
    ident_d = di("ident", [P, P])
    idx_d = di("idx", [T * BC], dt.int32)
    mf_d = di("mf", [P, T, BC], dt.uint8)
    mb_d = di("mb", [P, T, BC], dt.uint8)
    fcw_d = di("fcw", [2 * T, NCLS])
    fcb_d = di("fcb_rep", [BC, NCLS])
    wdram = {}
    for l in (0, 1):
        for d in ("f", "b"):
            wdram[f"wh{l}{d}"] = di(f"wh{l}{d}", [H, 4 * H])
            wdram[f"bcol{l}{d}"] = di(f"bcol{l}{d}", [P, 4])
            if l == 0:
                wdram[f"wx0{d}a"] = di(f"wx0{d}a", [E, 4 * H])
            else:
                wdram[f"wx1{d}f"] = di(f"wx1{d}f", [H, 4 * H])
                wdram[f"wx1{d}b"] = di(f"wx1{d}b", [H, 4 * H])

    out_d = nc.dram_tensor("out", [BC, NCLS], dt.float32, kind="ExternalOutput")

    # DRAM scratch: layer outputs (b-direction stored time-reversed)
    y_dram = {
        (l, d): nc.dram_tensor(f"y{l}{d}", [H, T, BC], dt.float32)
        for l in (0, 1) for d in ("f", "b")
    }
    feat_dram = nc.dram_tensor("feat", [2, T, BC], dt.float32)
    y1bf_dram = nc.dram_tensor("y1bf", [H, T, BC], dt.float32)

    NTOK = T * BC            # 4096 tokens per core
    NCH = NTOK // P          # 32 gather/pool chunks
    NXC = NTOK // 512        # 8 xp matmul chunks
    TCH = 512 // BC          # 64 timesteps per xp chunk
    KI, KF = -KSAT * 0.5, KSAT * 0.5  # post-colscale saturation constants

    with tile.TileContext(nc) as tc:
        with (
            tc.tile_pool(name="const", bufs=1) as cpool,
            tc.tile_pool(name="big", bufs=1) as bigpool,
            tc.tile_pool(name="work", bufs=4) as wpool,
            tc.tile_pool(name="psx", bufs=2, space="PSUM") as psx,
            tc.tile_pool(name="psz", bufs=4, space="PSUM") as psz,
            tc.tile_pool(name="psf", bufs=1, space="PSUM") as psf,
        ):
            # ---- constant loads
            ident = cpool.tile([P, P], dt.float32, tag="ident")
            nc.sync.dma_start(out=ident[:], in_=ident_d[:])
            idx_t = cpool.tile([P, NCH], dt.int32, tag="idx")
            nc.sync.dma_start(
                out=idx_t[:], in_=idx_d.rearrange("(c p) -> p c", p=P))
            masks = {}
            for d, md in (("f", mf_d), ("b", mb_d)):
                mt = cpool.tile([P, T, BC], dt.uint8, tag=f"m{d}", name=f"m{d}")
                nc.sync.dma_start(out=mt[:], in_=md[:])
                masks[d] = mt
            wsb = {}
            for k, dr in wdram.items():
                sh = list(dr.shape)
                wt = cpool.tile(sh, dt.float32, tag=k, name=k)
                nc.sync.dma_start(out=wt[:], in_=dr[:])
                wsb[k] = wt
            fcw_t = cpool.tile([P, 2 * T // P, NCLS], dt.float32, tag="fcw")
            nc.sync.dma_start(
                out=fcw_t[:], in_=fcw_d.rearrange("(q p) c -> p q c", p=P))
            fcb_t = cpool.tile([BC, NCLS], dt.float32, tag="fcb")
            nc.sync.dma_start(out=fcb_t[:], in_=fcb_d[:])

            xpT = {
                d: bigpool.tile([P, T, 4, BC], dt.float32, tag=f"xp{d}",
                                name=f"xp{d}")
                for d in ("f", "b")
            }

            def xp_epilogue(d, n, g, ps):
                """xpT[d][:, chunk, g, :] = ps + bias_col + K_g*(1-m)."""
                t0, t1 = n * TCH, (n + 1) * TCH
                dst = xpT[d][:, t0:t1, g, :]
                bcol = wsb[f"bcol{xp_epilogue.layer}{d}"]
                kg = KI if g == 0 else (KF if g == 1 else 0.0)
                if kg != 0.0:
                    # dst = m*(-kg) + ps  (then += bias + kg below)
                    nc.vector.scalar_tensor_tensor(
                        out=dst, in0=masks[d][:, t0:t1, :], scalar=-kg,
                        in1=ps[:], op0=ALU.mult, op1=ALU.add)
                    nc.vector.tensor_scalar(
                        out=dst, in0=dst, scalar1=bcol[:, g:g + 1],
                        scalar2=float(kg), op0=ALU.add, op1=ALU.add)
                else:
                    nc.vector.tensor_scalar(
                        out=dst, in0=ps[:], scalar1=bcol[:, g:g + 1],
                        scalar2=None, op0=ALU.add)

            # ---- embedding gather + transpose + layer-0 xp
            with tc.tile_pool(name="gph", bufs=3) as gpool, \
                 tc.tile_pool(name="gbig", bufs=1) as gbig:
                g128 = gbig.tile([P, T, BC], dt.float32, tag="g128")
                g128f = g128[:].rearrange("p t b -> p (t b)")
                for c in range(NCH):
                    gr = gpool.tile([P, E], dt.float32, tag="gr")
                    nc.gpsimd.indirect_dma_start(
                        out=gr[:], out_offset=None, in_=emb_d[:],
                        in_offset=bass.IndirectOffsetOnAxis(
                            ap=idx_t[:, c:c + 1], axis=0),
                    )
                    pt = psx.tile([P, P], dt.float32, tag="psxp")
                    nc.tensor.transpose(out=pt[:], in_=gr[:], identity=ident[:])
                    nc.vector.tensor_copy(
                        out=g128f[:, c * P:(c + 1) * P], in_=pt[:])

                xp_epilogue.layer = 0
                for d, rv in (("f", g128[:]), ("b", g128[:, ::-1, :])):
                    wxa = wsb[f"wx0{d}a"]
                    for n in range(NXC):
                        t0, t1 = n * TCH, (n + 1) * TCH
                        for g in range(4):
                            ps = psx.tile([P, TCH, BC], dt.float32, tag="psxp")
                            nc.tensor.matmul(
                                out=ps[:], lhsT=wxa[:, g * H:(g + 1) * H],
                                rhs=rv[:, t0:t1, :], start=True, stop=True)
                            xp_epilogue(d, n, g, ps)

            # ---- scan machinery
            Hs = {d: cpool.tile([P, BC], dt.float32, tag=f"H{d}", name=f"H{d}")
                  for d in "fb"}
            Cs = {d: cpool.tile([P, BC], dt.float32, tag=f"C{d}", name=f"C{d}")
                  for d in "fb"}

            def scan_layer(l):
                for d in "fb":
                    nc.vector.memset(Hs[d][:], 0.0)
                    nc.vector.memset(Cs[d][:], 0.0)
                wh = {d: wsb[f"wh{l}{d}"] for d in "fb"}
                with tc.For_i(0, T, UNROLL) as t0:
                    for j in range(UNROLL):
                        for d in "fb":
                            Hd, Cd = Hs[d], Cs[d]
                            zp = psz.tile([P, 4, BC], dt.float32, tag="zp")
                            for g in range(4):
                                nc.tensor.matmul(
                                    out=zp[:, g, :],
                                    lhsT=wh[d][:, g * H:(g + 1) * H],
                                    rhs=Hd[:], start=True, stop=True)
                            zs = wpool.tile([P, 4, BC], dt.float32, tag="zs")
                            nc.vector.tensor_tensor(
                                out=zs[:],
                                in0=xpT[d][:, bass.ds(t0 + j, 1), :, :],
                                in1=zp[:],
                                op=ALU.add)
                            tall = wpool.tile([P, 4, BC], dt.float32, tag="tall")
                            nc.scalar.activation(
                                out=tall[:], in_=zs[:], func=AF.Tanh)
                            wt = wpool.tile([P, BC], dt.float32, tag="wt")
                            nc.vector.scalar_tensor_tensor(
                                out=wt[:], in0=tall[:, 0, :], scalar=1.0,
                                in1=tall[:, 2, :], op0=ALU.add, op1=ALU.mult)
                            pt_ = wpool.tile([P, BC], dt.float32, tag="pt")
                            nc.vector.scalar_tensor_tensor(
                                out=pt_[:], in0=tall[:, 1, :], scalar=1.0,
                                in1=Cd[:], op0=ALU.add, op1=ALU.mult)
                            nc.vector.scalar_tensor_tensor(
                                out=Cd[:], in0=pt_[:], scalar=0.5,
                                in1=wt[:], op0=ALU.mult, op1=ALU.add)
                            tct = wpool.tile([P, BC], dt.float32, tag="tct")
                            nc.scalar.activation(
                                out=tct[:], in_=Cd[:], func=AF.Tanh, scale=0.5)
                            rt = wpool.tile([P, BC], dt.float32, tag="rt")
                            nc.vector.scalar_tensor_tensor(
                                out=rt[:], in0=tall[:, 3, :], scalar=1.0,
                                in1=tct[:], op0=ALU.add, op1=ALU.mult)
                            nc.vector.copy_predicated(
                                out=Hd[:],
                                mask=masks[d][:, bass.ds(t0 + j, 1), :],
                                data=rt[:])
                            stg = wpool.tile([P, BC], dt.float32, tag="stg")
                            nc.vector.tensor_copy(out=stg[:], in_=Hd[:])
                            nc.sync.dma_start(
                                out=y_dram[(l, d)][:, bass.ds(t0 + j, 1), :],
                                in_=stg[:])

            scan_layer(0)

            # ---- layer-1 xp from DRAM y0 (time views per direction)
            views = {
                "f": (y_dram[(0, "f")][:], y_dram[(0, "b")][:, ::-1, :]),
                "b": (y_dram[(0, "f")][:, ::-1, :], y_dram[(0, "b")][:]),
            }
            xp_epilogue.layer = 1
            with tc.tile_pool(name="s1", bufs=2) as spool1:
                for d in "fb":
                    vf, vb = views[d]
                    for n in range(NXC):
                        t0, t1 = n * TCH, (n + 1) * TCH
                        sf = spool1.tile([P, TCH, BC], dt.float32, tag="sf")
                        nc.sync.dma_start(out=sf[:], in_=vf[:, t0:t1, :])
                        sb_ = spool1.tile([P, TCH, BC], dt.float32, tag="sb")
                        nc.sync.dma_start(out=sb_[:], in_=vb[:, t0:t1, :])
                        for g in range(4):
                            ps = psx.tile([P, TCH, BC], dt.float32, tag="psxp")
                            nc.tensor.matmul(
                                out=ps[:],
                                lhsT=wsb[f"wx1{d}f"][:, g * H:(g + 1) * H],
                                rhs=sf[:], start=True, stop=False)
                            nc.tensor.matmul(
                                out=ps[:],
                                lhsT=wsb[f"wx1{d}b"][:, g * H:(g + 1) * H],
                                rhs=sb_[:], start=False, stop=True)
                            xp_epilogue(d, n, g, ps)

            scan_layer(1)

            # ---- pooling over the 256 concat features per token
            with tc.tile_pool(name="ep", bufs=3) as epool:
                fmx = cpool.tile([P, NCH], dt.float32, tag="fmx")
                fsum = cpool.tile([P, NCH], dt.float32, tag="fsum")
                # un-reverse y1b into forward-time DRAM (dram->dram DMA,
                # chunked: walrus caps AP dim counts at 16 bits)
                yrev = y_dram[(1, "b")][:, ::-1, :]
                for rc in range(8):
                    nc.sync.dma_start(
                        out=y1bf_dram[:, rc * 64:(rc + 1) * 64, :],
                        in_=yrev[:, rc * 64:(rc + 1) * 64, :])
                y1f_tok = y_dram[(1, "f")].rearrange("h t b -> (t b) h")
                y1b_tok = y1bf_dram.rearrange("h t b -> (t b) h")
                for c in range(NCH):
                    ycat = epool.tile([P, 2 * H], dt.float32, tag="ycat")
                    nc.sync.dma_start(
                        out=ycat[:, 0:H], in_=y1f_tok[c * P:(c + 1) * P, :])
                    nc.sync.dma_start(
                        out=ycat[:, H:2 * H], in_=y1b_tok[c * P:(c + 1) * P, :])
                    nc.vector.tensor_reduce(
                        out=fmx[:, c:c + 1], in_=ycat[:],
                        axis=mybir.AxisListType.XYZW, op=ALU.max)
                    nc.vector.tensor_reduce(
                        out=fsum[:, c:c + 1], in_=ycat[:],
                        axis=mybir.AxisListType.XYZW, op=ALU.add)
                featv = feat_dram.rearrange("s t b -> s (t b)")
                nc.sync.dma_start(
                    out=featv[0].rearrange("(c p) -> p c", p=P), in_=fmx[:])
                nc.sync.dma_start(
                    out=featv[1].rearrange("(c p) -> p c", p=P), in_=fsum[:])

                # ---- FC head: out = relu(featT.T @ fcw + b)
                pfc = psf.tile([BC, NCLS], dt.float32, tag="pfc")
                NQ = 2 * T // P
                for q in range(NQ):
                    lq = epool.tile([P, BC], dt.float32, tag="lq")
                    pool_i, tq = divmod(q * P, T)
                    nc.sync.dma_start(
                        out=lq[:], in_=feat_dram[pool_i, tq:tq + P, :])
                    nc.tensor.matmul(
                        out=pfc[:], lhsT=lq[:], rhs=fcw_t[:, q, :],
                        start=(q == 0), stop=(q == NQ - 1))
                ob = epool.tile([BC, NCLS], dt.float32, tag="ob")
                nc.vector.tensor_tensor(
                    out=ob[:], in0=pfc[:], in1=fcb_t[:], op=ALU.add)
                nc.vector.tensor_scalar(
                    out=ob[:], in0=ob[:], scalar1=0.0, scalar2=None,
                    op0=ALU.max)
                nc.sync.dma_start(out=out_d[:], in_=ob[:])

    split_multi_waits(nc)
    return nc


_cached_nc = None


def _get_nc():
    global _cached_nc
    if _cached_nc is None:
        _install_hook()
        _cached_nc = _build()
    return _cached_nc


def _in_maps(inputs):
    w = _fold_weights(inputs)
    x = np.asarray(inputs["x"]).astype(np.int32)  # [64, 512]
    shared = {
        "emb": w["emb"], "ident": w["ident"], "fcw": w["fcw"],
        "fcb_rep": w["fcb_rep"],
    }
    for l in (0, 1):
        for d in ("f", "b"):
            shared[f"wh{l}{d}"] = w[f"wh{l}{d}"]
            shared[f"bcol{l}{d}"] = w[f"bcol{l}{d}"]
            if l == 0:
                shared[f"wx0{d}a"] = w[f"wx0{d}a"]
            else:
                shared[f"wx1{d}f"] = w[f"wx1{d}f"]
                shared[f"wx1{d}b"] = w[f"wx1{d}b"]
    maps = []
    for c in range(NCORES):
        xc = x[c * BC:(c + 1) * BC]            # [BC, T]
        idx = np.ascontiguousarray(xc.T).reshape(-1).astype(np.int32)
        m = (xc != 0).astype(np.uint8).T       # [T, BC]
        mf = np.broadcast_to(m[None], (P, T, BC))
        mb = mf[:, ::-1, :]
        maps.append(dict(shared, idx=idx,
                         mf=np.ascontiguousarray(mf),
                         mb=np.ascontiguousarray(mb)))
    return maps


def _run(inputs, trace=False):
    from concourse.bass_utils import run_bass_kernel_spmd
    nc = _get_nc()
    maps = _in_maps(inputs)
    res = run_bass_kernel_spmd(nc, maps, list(range(NCORES)), trace=trace)
    out = np.concatenate([res.results[c]["out"] for c in range(NCORES)], axis=0)
    return out.astype(np.float32), res


def kernel(**inputs):
    out, _ = _run(inputs, trace=False)
    return out


def run_traced(inputs):
    out, res = _run(inputs, trace=True)
    return out, res



# revision 8
# speedup vs baseline: 4.2431x; 4.2431x over previous
"""Bass/TRN2 kernel for nn_BiRNNLayers: 2-layer BiLSTM (B=64, T=512, H=128,
vocab 50000) with masked Keras-style scan, feature pooling and FC head.

v2 strategy (8 NeuronCores, data-parallel over batch, 8 rows/core):
- fp16 weights/activations for the recurrent path (fast LDWEIGHTS+MATMUL,
  2x DVE), fp32 cell state and gate math.
- One tanh table for everything: every activation is Tanh(scale=0.5); the
  g-gate columns are pre-scaled by 2 so tanh(z_g) comes out right. No
  ACT_TABLE_LOAD thrash.
- xp (input projections incl. bias and +-K mask saturation) precomputed
  into SBUF fp16; per step it is preloaded into PSUM with an identity
  matmul and the 4 recurrent gate matmuls accumulate on top.
- State kept as H'=2h (fp16, in the y tile) and C'=2c (fp32). Cell math:
  tanh gates -> fused (t+1)*x ops on DVE, h' on GpSimd, masked-carry
  copy_predicated on DVE. y stays SBUF-resident; no per-step DMA.
- For_i with UNROLL=32 to amortize the tile-loop barrier.
- Layer-1 xp read straight from SBUF y0 (time-reversed views for the
  opposite direction); pooling via PE transposes of SBUF y1 + DVE
  reduces; FC through a small DRAM bounce for the transposed layout.
"""
import os
import numpy as np

import concourse.bass as bass
import concourse.mybir as mybir
import concourse.tile as tile
import bass_rust

P = 128
T = 512
H = 128
E = 128
B_FULL = 64
NCORES = 8
BC = B_FULL // NCORES  # batch rows per core
VOCAB = 50000
NCLS = 10
KSAT = 40.0            # pre-activation saturation offset for masked steps
UNROLL = 32
DBG = os.environ.get("KDBG", "0") == "1"

AF = mybir.ActivationFunctionType
ALU = mybir.AluOpType
dt = mybir.dt

_hook_installed = False


def _install_hook():
    """Surface compile-hook tracebacks (PJRT swallows them otherwise)."""
    global _hook_installed
    if _hook_installed:
        return
    _hook_installed = True
    import traceback
    import concourse.bass2jax as bass2jax
    import libneuronxla

    orig = bass2jax.neuronx_cc_hook

    def dbg_hook(*a, **k):
        try:
            return orig(*a, **k)
        except BaseException:
            traceback.print_exc()
            raise

    bass2jax.neuronx_cc_hook = dbg_hook
    if not hasattr(libneuronxla, "orig_neuronx_cc"):
        libneuronxla.orig_neuronx_cc = libneuronxla.neuronx_cc
    libneuronxla.neuronx_cc = dbg_hook


def split_multi_waits(nc):
    """This container's walrus encodes at most one sem wait per instruction;
    hoist extra waits onto preceding same-engine NoOps."""
    for fn in nc.m.functions:
        for bb in fn.blocks:
            out = []
            changed = False
            for inst in bb.instructions:
                si = inst.sync_info
                waits = list(si.on_wait) if si is not None and si.on_wait else []
                if len(waits) > 1:
                    changed = True
                    for k, w in enumerate(waits[:-1]):
                        nop = mybir.InstNoOp(name=f"{inst.name}-sw{k}")
                        nop.engine = inst.engine
                        nop.sync_info = bass_rust.SyncInfo(on_wait=[w], on_update=[])
                        out.append(nop)
                    inst.sync_info = bass_rust.SyncInfo(
                        on_wait=[waits[-1]], on_update=list(si.on_update)
                    )
                out.append(inst)
            if changed:
                bb.instructions = out


# ---------------------------------------------------------------------------
# host-side weight folding
# ---------------------------------------------------------------------------

def _fold_weights(inputs):
    f32, f16 = np.float32, np.float16
    # every activation is tanh(0.5*z'): i,f,o columns raw (sigmoid =
    # (tanh(z/2)+1)/2), g columns pre-scaled by 2 so tanh(0.5*2*z)=tanh(z).
    cs = np.concatenate([
        np.ones(H), np.ones(H), np.full(H, 2.0), np.ones(H)
    ]).astype(f32)

    w = {}
    for l in (0, 1):
        for d in ("f", "b"):
            Wx = np.asarray(inputs[f"Wx_{d}{l}"], f32)
            Wh = np.asarray(inputs[f"Wh_{d}{l}"], f32)
            b = np.asarray(inputs[f"b_{d}{l}"], f32)
            # recurrent input is H'=2h -> fold 0.5 into Wh
            w[f"wh{l}{d}"] = ((Wh * 0.5) * cs).astype(f16)
            be = (b * cs).astype(f32)
            w[f"bcol{l}{d}"] = np.ascontiguousarray(
                be.reshape(4, H).T)  # [128, 4] f32
            if l == 0:
                w[f"wx0{d}"] = (Wx * cs).astype(f16)
            else:
                w[f"wx1{d}f"] = ((Wx[0:H] * 0.5) * cs).astype(f16)
                w[f"wx1{d}b"] = ((Wx[H:2 * H] * 0.5) * cs).astype(f16)

    w["emb"] = np.asarray(inputs["emb"], f32)

    fcw = np.asarray(inputs["fc_W"], f32).copy()  # [2T, 10]
    fcw[:T] *= 0.5          # mx rows: feat carries 2*mx
    fcw[T:] *= 1.0 / 512.0  # av rows: feat carries sum(2h) over 256 feats
    w["fcw"] = fcw.astype(f32)
    w["fcb_rep"] = np.tile(np.asarray(inputs["fc_b"], f32)[None, :], (BC, 1))
    w["identf"] = np.eye(P, dtype=f32)
    w["identh"] = np.eye(P, dtype=f16)
    # saturation rows: [1, 2H]: first H = -K (i gate), second H = +K (f gate)
    w["satrow"] = np.concatenate(
        [np.full(H, -KSAT, f16), np.full(H, KSAT, f16)])[None, :]
    return w


# ---------------------------------------------------------------------------
# device program
# ---------------------------------------------------------------------------

def _build():
    nc = bass.Bass("TRN2", target_bir_lowering=False, debug=False,
                   num_devices=NCORES)

    def di(name, shape, dtype=dt.float32):
        return nc.dram_tensor(name, shape, dtype, kind="ExternalInput")

    emb_d = di("emb", [VOCAB + 1, E])
    identf_d = di("identf", [P, P])
    identh_d = di("identh", [P, P], dt.float16)
    satrow_d = di("satrow", [1, 2 * H], dt.float16)
    idx_d = di("idx", [T * BC], dt.int32)
    minv_d = di("minv", [P, T, 2, BC], dt.uint8)
    minvmm_d = di("minvmm", [1, T, 2, BC], dt.float16)
    fcw_d = di("fcw", [2 * T, NCLS])
    fcb_d = di("fcb_rep", [BC, NCLS])
    wdram = {}
    for l in (0, 1):
        for d in ("f", "b"):
            wdram[f"wh{l}{d}"] = di(f"wh{l}{d}", [H, 4 * H], dt.float16)
            wdram[f"bcol{l}{d}"] = di(f"bcol{l}{d}", [P, 4])
            if l == 0:
                wdram[f"wx0{d}"] = di(f"wx0{d}", [E, 4 * H], dt.float16)
            else:
                wdram[f"wx1{d}f"] = di(f"wx1{d}f", [H, 4 * H], dt.float16)
                wdram[f"wx1{d}b"] = di(f"wx1{d}b", [H, 4 * H], dt.float16)

    out_d = nc.dram_tensor("out", [BC, NCLS], dt.float32, kind="ExternalOutput")
    feat_dram = nc.dram_tensor("feat", [2, T, BC], dt.float32)
    dbg = {}
    if DBG:
        for nm, sh, dty in [
            ("dxp0", [P, T, 2, 4, BC], dt.float16),
            ("dy0f", [P, T + 1, BC], dt.float16),
            ("dy0b", [P, T + 1, BC], dt.float16),
            ("dy1f", [P, T + 1, BC], dt.float16),
            ("dy1b", [P, T + 1, BC], dt.float16),
            ("dfmx", [P, 32], dt.float32),
            ("dfsum", [P, 32], dt.float32),
        ]:
            dbg[nm] = nc.dram_tensor(nm, sh, dty, kind="ExternalOutput")

    NTOK = T * BC            # 4096 tokens per core
    NCH = NTOK // P          # 32 gather/pool chunks
    NXC = 8                  # xp matmul chunks
    TCH = T // NXC           # 64 timesteps per xp chunk

    with tile.TileContext(nc) as tc:
        with (
            tc.tile_pool(name="const", bufs=1) as cpool,
            tc.tile_pool(name="big", bufs=1) as bigpool,
            tc.tile_pool(name="psz", bufs=4, space="PSUM") as psz,
        ):
            # ---- constant loads
            identf = cpool.tile([P, P], dt.float32, tag="identf")
            nc.sync.dma_start(out=identf[:], in_=identf_d[:])
            identh = cpool.tile([P, P], dt.float16, tag="identh")
            nc.sync.dma_start(out=identh[:], in_=identh_d[:])
            satrow = cpool.tile([1, 2 * H], dt.float16, tag="satrow")
            nc.sync.dma_start(out=satrow[:], in_=satrow_d[:])
            idx_t = cpool.tile([P, NCH], dt.int32, tag="idx")
            nc.sync.dma_start(
                out=idx_t[:], in_=idx_d.rearrange("(c p) -> p c", p=P))
            minv = cpool.tile([P, T, 2, BC], dt.uint8, tag="minv", name="minv")
            nc.sync.dma_start(out=minv[:], in_=minv_d[:])
            minvmm = cpool.tile([1, T, 2, BC], dt.float16, tag="minvmm",
                                name="minvmm")
            nc.sync.dma_start(out=minvmm[:], in_=minvmm_d[:])
            wsb = {}
            for k, dr in wdram.items():
                sh = list(dr.shape)
                wt = cpool.tile(sh, dr.dtype, tag=k, name=k)
                nc.sync.dma_start(out=wt[:], in_=dr[:])
                wsb[k] = wt
            fcw_t = cpool.tile([P, 2 * T // P, NCLS], dt.float32, tag="fcw")
            nc.sync.dma_start(
                out=fcw_t[:], in_=fcw_d.rearrange("(q p) c -> p q c", p=P))
            fcb_t = cpool.tile([BC, NCLS], dt.float32, tag="fcb")
            nc.sync.dma_start(out=fcb_t[:], in_=fcb_d[:])

            # big SBUF-resident tensors
            xp = bigpool.tile([P, T, 2, 4, BC], dt.float16, tag="xp", name="xp")
            y0 = {d: bigpool.tile([P, T + 1, BC], dt.float16,
                                  tag=f"y0{d}", name=f"y0{d}") for d in "fb"}
            y1 = {d: bigpool.tile([P, T + 1, BC], dt.float16,
                                  tag=f"y1{d}", name=f"y1{d}") for d in "fb"}
            g128 = bigpool.tile([P, T, BC], dt.float16, tag="g128", name="g128")

            # ---- embedding gather -> fp16 token matrix (h on partitions)
            g128f = g128[:].rearrange("p t b -> p (t b)")
            with tc.tile_pool(name="gph", bufs=3) as gpool, \
                 tc.tile_pool(name="psg", bufs=2, space="PSUM") as psg:
                for c in range(NCH):
                    gr = gpool.tile([P, E], dt.float32, tag="gr")
                    nc.gpsimd.indirect_dma_start(
                        out=gr[:], out_offset=None, in_=emb_d[:],
                        in_offset=bass.IndirectOffsetOnAxis(
                            ap=idx_t[:, c:c + 1], axis=0),
                    )
                    pt = psg.tile([P, P], dt.float32, tag="psg")
                    nc.tensor.transpose(out=pt[:], in_=gr[:], identity=identf[:])
                    nc.vector.tensor_copy(
                        out=g128f[:, c * P:(c + 1) * P], in_=pt[:])

            def xp_chunks(l, srcs):
                """Compute xp for layer l.

                srcs[d] = list of (lhsT_tile_key, rhs_view) accumulated per
                gate; bias added during PSUM->SBUF evac; +-K saturation for
                masked steps enters via a K=1 matmul against (1-mask).
                """
                with tc.tile_pool(name=f"psx{l}", bufs=3, space="PSUM") as psx:
                    for di_, d in enumerate("fb"):
                        bcol = wsb[f"bcol{l}{d}"]
                        for n in range(NXC):
                            t0, t1 = n * TCH, (n + 1) * TCH
                            for g in range(4):
                                px = psx.tile([P, TCH, BC], dt.float32,
                                              tag="px")
                                terms = srcs[d]
                                sat = g in (0, 1)
                                for ti_, (wkey, view) in enumerate(terms):
                                    last = (ti_ == len(terms) - 1) and not sat
                                    nc.tensor.matmul(
                                        out=px[:],
                                        lhsT=wsb[wkey][:, g * H:(g + 1) * H],
                                        rhs=view[:, t0:t1, :],
                                        start=(ti_ == 0), stop=last)
                                if sat:
                                    nc.tensor.matmul(
                                        out=px[:],
                                        lhsT=satrow[:, g * H:(g + 1) * H],
                                        rhs=minvmm[:, t0:t1, di_, :],
                                        start=False, stop=True)
                                nc.vector.tensor_scalar(
                                    out=xp[:, t0:t1, di_, g, :], in0=px[:],
                                    scalar1=bcol[:, g:g + 1], scalar2=None,
                                    op0=ALU.add)

            # ---- layer-0 xp from embeddings
            g3 = g128[:]  # [P, T, BC] view
            xp_chunks(0, {
                "f": [(f"wx0f", g3)],
                "b": [(f"wx0b", g3[:, ::-1, :])],
            })

            # ---- scan machinery
            SB = {d: cpool.tile([P, 9, BC], dt.float32, tag=f"SB{d}",
                                name=f"SB{d}")
                  for d in "fb"}

            def scan_layer(l, y):
                for di_, d in enumerate("fb"):
                    nc.vector.memset(SB[d][:, 4, :], 0.0)   # C'
                    nc.vector.memset(y[d][:, 0, :], 0.0)    # H'
                wh = {d: wsb[f"wh{l}{d}"] for d in "fb"}
                if True:  # fully static unroll over time
                    for tj in range(T):
                        tj1 = tj + 1
                        for di_, d in enumerate("fb"):
                            S = SB[d]
                            zp = psz.tile([P, 4, BC], dt.float32, tag="zp")
                            nc.tensor.matmul(
                                out=zp[:], lhsT=identh[:],
                                rhs=xp[:, tj, di_, :, :],
                                start=True, stop=False, skip_group_check=True)
                            for g in range(4):
                                nc.tensor.matmul(
                                    out=zp[:, g, :],
                                    lhsT=wh[d][:, g * H:(g + 1) * H],
                                    rhs=y[d][:, tj, :],
                                    start=False, stop=True,
                                    skip_group_check=True)
                            nc.scalar.activation(
                                out=S[:, 0:4, :], in_=zp[:], func=AF.Tanh,
                                scale=0.5)
                            # wt = (ti+1)*tg ; pt = (tf+1)*C'
                            nc.vector.scalar_tensor_tensor(
                                out=S[:, 5:7, :], in0=S[:, 0:2, :], scalar=1.0,
                                in1=S[:, 2:5:2, :], op0=ALU.add, op1=ALU.mult)
                            # C' = 0.5*pt + wt
                            nc.vector.scalar_tensor_tensor(
                                out=S[:, 4, :], in0=S[:, 6, :], scalar=0.5,
                                in1=S[:, 5, :], op0=ALU.mult, op1=ALU.add)
                            # tct = tanh(0.5*C')
                            nc.scalar.activation(
                                out=S[:, 7, :], in_=S[:, 4, :], func=AF.Tanh,
                                scale=0.5)
                            # H' = (to+1)*tct -> y[t+1] (Pool only has
                            # 2-operand tensor_tensor: to*tct, then +tct)
                            nc.gpsimd.tensor_tensor(
                                out=S[:, 8, :], in0=S[:, 3, :],
                                in1=S[:, 7, :], op=ALU.mult)
                            nc.gpsimd.tensor_tensor(
                                out=y[d][:, tj1, :], in0=S[:, 8, :],
                                in1=S[:, 7, :], op=ALU.add)
                            # masked steps carry previous H'
                            nc.vector.copy_predicated(
                                out=y[d][:, tj1, :],
                                mask=minv[:, tj, di_, :],
                                data=y[d][:, tj, :])

            if DBG:
                nc.sync.dma_start(out=dbg["dxp0"][:], in_=xp[:])
            scan_layer(0, y0)
            if DBG:
                nc.sync.dma_start(out=dbg["dy0f"][:], in_=y0["f"][:])
                nc.sync.dma_start(out=dbg["dy0b"][:], in_=y0["b"][:])

            # ---- layer-1 xp from SBUF y0
            yf = y0["f"][:, 1:T + 1, :]
            yb = y0["b"][:, 1:T + 1, :]
            yfr = yf[:, ::-1, :]
            ybr = yb[:, ::-1, :]
            xp_chunks(1, {
                "f": [("wx1ff", yf), ("wx1fb", ybr)],
                "b": [("wx1bf", yfr), ("wx1bb", yb)],
            })

            scan_layer(1, y1)
            if DBG:
                nc.sync.dma_start(out=dbg["dy1f"][:], in_=y1["f"][:])
                nc.sync.dma_start(out=dbg["dy1b"][:], in_=y1["b"][:])

            # ---- pooling over the 256 concat features per token
            fmx = cpool.tile([P, NCH], dt.float32, tag="fmx")
            fsum = cpool.tile([P, NCH], dt.float32, tag="fsum")
            yb_nat = y1["b"][:, 1:T + 1, :][:, ::-1, :]  # natural time
            with tc.tile_pool(name="psp", bufs=2, space="PSUM") as psp, \
                 tc.tile_pool(name="ystg", bufs=3) as ystgp:
                for c in range(NCH):
                    # stage to fp32 SBUF (the copy also un-reverses the
                    # scan-step-major b direction), transpose fp32, reduce.
                    ystg = ystgp.tile([P, 2, 16, BC], dt.float32, tag="ystg")
                    pt = psp.tile([P, 2, P], dt.float32, tag="pt")
                    srcs = (y1["f"][:, 1 + 16 * c:1 + 16 * (c + 1), :],
                            yb_nat[:, 16 * c:16 * (c + 1), :])
                    for di_ in range(2):
                        nc.gpsimd.tensor_copy(
                            out=ystg[:, di_, :, :], in_=srcs[di_])
                        nc.tensor.transpose(
                            out=pt[:, di_, :],
                            in_=ystg[:, di_, :, :],
                            identity=identf[:])
                    nc.vector.tensor_reduce(
                        out=fmx[:, c:c + 1], in_=pt[:],
                        axis=mybir.AxisListType.XYZW, op=ALU.max)
                    nc.vector.tensor_reduce(
                        out=fsum[:, c:c + 1], in_=pt[:],
                        axis=mybir.AxisListType.XYZW, op=ALU.add)

            if DBG:
                nc.sync.dma_start(out=dbg["dfmx"][:], in_=fmx[:])
                nc.sync.dma_start(out=dbg["dfsum"][:], in_=fsum[:])

            # ---- FC head via small DRAM bounce (transposed feat layout)
            with tc.tile_pool(name="ep", bufs=3) as epool, \
                 tc.tile_pool(name="psf", bufs=1, space="PSUM") as psf:
                featv = feat_dram  # [2, T, BC]
                nc.sync.dma_start(
                    out=featv[0].rearrange("(c q) b -> (q b) c", q=16),
                    in_=fmx[:])
                nc.sync.dma_start(
                    out=featv[1].rearrange("(c q) b -> (q b) c", q=16),
                    in_=fsum[:])
                pfc = psf.tile([BC, NCLS], dt.float32, tag="pfc")
                NQ = 2 * T // P
                for q in range(NQ):
                    lq = epool.tile([P, BC], dt.float32, tag="lq")
                    pool_i, tq = divmod(q * P, T)
                    nc.sync.dma_start(
                        out=lq[:], in_=feat_dram[pool_i, tq:tq + P, :])
                    nc.tensor.matmul(
                        out=pfc[:], lhsT=lq[:], rhs=fcw_t[:, q, :],
                        start=(q == 0), stop=(q == NQ - 1))
                ob = epool.tile([BC, NCLS], dt.float32, tag="ob")
                nc.vector.tensor_tensor(
                    out=ob[:], in0=pfc[:], in1=fcb_t[:], op=ALU.add)
                nc.vector.tensor_scalar(
                    out=ob[:], in0=ob[:], scalar1=0.0, scalar2=None,
                    op0=ALU.max)
                nc.sync.dma_start(out=out_d[:], in_=ob[:])

    split_multi_waits(nc)
    return nc


_cached_nc = None


def _get_nc():
    global _cached_nc
    if _cached_nc is None:
        _install_hook()
        _cached_nc = _build()
    return _cached_nc


def _in_maps(inputs):
    w = _fold_weights(inputs)
    x = np.asarray(inputs["x"]).astype(np.int32)  # [64, 512]
    shared = {
        "emb": w["emb"], "identf": w["identf"], "identh": w["identh"],
        "satrow": w["satrow"], "fcw": w["fcw"], "fcb_rep": w["fcb_rep"],
    }
    for l in (0, 1):
        for d in ("f", "b"):
            shared[f"wh{l}{d}"] = w[f"wh{l}{d}"]
            shared[f"bcol{l}{d}"] = w[f"bcol{l}{d}"]
            if l == 0:
                shared[f"wx0{d}"] = w[f"wx0{d}"]
            else:
                shared[f"wx1{d}f"] = w[f"wx1{d}f"]
                shared[f"wx1{d}b"] = w[f"wx1{d}b"]
    maps = []
    for c in range(NCORES):
        xc = x[c * BC:(c + 1) * BC]            # [BC, T]
        idx = np.ascontiguousarray(xc.T).reshape(-1).astype(np.int32)
        m = (xc != 0).T                        # [T, BC] bool
        minv_f = (~m).astype(np.uint8)         # forward-time inverted
        minv_b = minv_f[::-1]                  # scan-step s <-> t = T-1-s
        mi = np.stack([minv_f, minv_b], axis=1)  # [T, 2, BC]
        minv = np.broadcast_to(mi[None], (P, T, 2, BC))
        maps.append(dict(
            shared, idx=idx,
            minv=np.ascontiguousarray(minv),
            minvmm=np.ascontiguousarray(mi[None]).astype(np.float16),
        ))
    return maps


def _run(inputs, trace=False):
    from concourse.bass_utils import run_bass_kernel_spmd
    nc = _get_nc()
    maps = _in_maps(inputs)
    res = run_bass_kernel_spmd(nc, maps, list(range(NCORES)), trace=trace)
    out = np.concatenate([res.results[c]["out"] for c in range(NCORES)], axis=0)
    return out.astype(np.float32), res


def kernel(**inputs):
    out, _ = _run(inputs, trace=False)
    return out


def run_traced(inputs):
    out, res = _run(inputs, trace=True)
    return out, res


# revision 9
# speedup vs baseline: 5.1370x; 1.2107x over previous
"""Bass/TRN2 kernel for nn_BiRNNLayers: 2-layer BiLSTM (B=64, T=512, H=128,
vocab 50000) with masked Keras-style scan, feature pooling and FC head.

v2 strategy (8 NeuronCores, data-parallel over batch, 8 rows/core):
- fp16 weights/activations for the recurrent path (fast LDWEIGHTS+MATMUL,
  2x DVE), fp32 cell state and gate math.
- One tanh table for everything: every activation is Tanh(scale=0.5); the
  g-gate columns are pre-scaled by 2 so tanh(z_g) comes out right. No
  ACT_TABLE_LOAD thrash.
- xp (input projections incl. bias and +-K mask saturation) precomputed
  into SBUF fp16; per step it is preloaded into PSUM with an identity
  matmul and the 4 recurrent gate matmuls accumulate on top.
- State kept as H'=2h (fp16, in the y tile) and C'=2c (fp32). Cell math:
  tanh gates -> fused (t+1)*x ops on DVE, h' on GpSimd, masked-carry
  copy_predicated on DVE. y stays SBUF-resident; no per-step DMA.
- For_i with UNROLL=32 to amortize the tile-loop barrier.
- Layer-1 xp read straight from SBUF y0 (time-reversed views for the
  opposite direction); pooling via PE transposes of SBUF y1 + DVE
  reduces; FC through a small DRAM bounce for the transposed layout.
"""
import os
import numpy as np

import concourse.bass as bass
import concourse.mybir as mybir
import concourse.tile as tile
import bass_rust

P = 128
T = 512
H = 128
E = 128
B_FULL = 64
NCORES = 8
BC = B_FULL // NCORES  # batch rows per core
VOCAB = 50000
NCLS = 10
KSAT = 40.0            # pre-activation saturation offset for masked steps
UNROLL = 32
DBG = os.environ.get("KDBG", "0") == "1"

AF = mybir.ActivationFunctionType
ALU = mybir.AluOpType
dt = mybir.dt

_hook_installed = False


def _install_hook():
    """Surface compile-hook tracebacks (PJRT swallows them otherwise)."""
    global _hook_installed
    if _hook_installed:
        return
    _hook_installed = True
    import traceback
    import concourse.bass2jax as bass2jax
    import libneuronxla

    orig = bass2jax.neuronx_cc_hook

    def dbg_hook(*a, **k):
        try:
            return orig(*a, **k)
        except BaseException:
            traceback.print_exc()
            raise

    bass2jax.neuronx_cc_hook = dbg_hook
    if not hasattr(libneuronxla, "orig_neuronx_cc"):
        libneuronxla.orig_neuronx_cc = libneuronxla.neuronx_cc
    libneuronxla.neuronx_cc = dbg_hook


def split_multi_waits(nc):
    """This container's walrus encodes at most one sem wait per instruction;
    hoist extra waits onto preceding same-engine NoOps."""
    for fn in nc.m.functions:
        for bb in fn.blocks:
            out = []
            changed = False
            for inst in bb.instructions:
                si = inst.sync_info
                waits = list(si.on_wait) if si is not None and si.on_wait else []
                if len(waits) > 1:
                    changed = True
                    for k, w in enumerate(waits[:-1]):
                        nop = mybir.InstNoOp(name=f"{inst.name}-sw{k}")
                        nop.engine = inst.engine
                        nop.sync_info = bass_rust.SyncInfo(on_wait=[w], on_update=[])
                        out.append(nop)
                    inst.sync_info = bass_rust.SyncInfo(
                        on_wait=[waits[-1]], on_update=list(si.on_update)
                    )
                out.append(inst)
            if changed:
                bb.instructions = out


# ---------------------------------------------------------------------------
# host-side weight folding
# ---------------------------------------------------------------------------

def _fold_weights(inputs):
    f32, f16 = np.float32, np.float16
    # every activation is tanh(0.5*z'): i,f,o columns raw (sigmoid =
    # (tanh(z/2)+1)/2), g columns pre-scaled by 2 so tanh(0.5*2*z)=tanh(z).
    cs = np.concatenate([
        np.ones(H), np.ones(H), np.full(H, 2.0), np.ones(H)
    ]).astype(f32)

    w = {}
    for l in (0, 1):
        for d in ("f", "b"):
            Wx = np.asarray(inputs[f"Wx_{d}{l}"], f32)
            Wh = np.asarray(inputs[f"Wh_{d}{l}"], f32)
            b = np.asarray(inputs[f"b_{d}{l}"], f32)
            # recurrent input is H'=2h -> fold 0.5 into Wh
            w[f"wh{l}{d}"] = ((Wh * 0.5) * cs).astype(f16)
            be = (b * cs).astype(f32)
            w[f"bcol{l}{d}"] = np.ascontiguousarray(
                be.reshape(4, H).T)  # [128, 4] f32
            if l == 0:
                w[f"wx0{d}"] = (Wx * cs).astype(f16)
            else:
                w[f"wx1{d}f"] = ((Wx[0:H] * 0.5) * cs).astype(f16)
                w[f"wx1{d}b"] = ((Wx[H:2 * H] * 0.5) * cs).astype(f16)

    w["emb"] = np.asarray(inputs["emb"], f32)

    fcw = np.asarray(inputs["fc_W"], f32).copy()  # [2T, 10]
    fcw[:T] *= 0.5          # mx rows: feat carries 2*mx
    fcw[T:] *= 1.0 / 512.0  # av rows: feat carries sum(2h) over 256 feats
    w["fcw"] = fcw.astype(f32)
    w["fcb_rep"] = np.tile(np.asarray(inputs["fc_b"], f32)[None, :], (BC, 1))
    w["identf"] = np.eye(P, dtype=f32)
    w["identh"] = np.eye(P, dtype=f16)
    # saturation rows: [1, 2H]: first H = -K (i gate), second H = +K (f gate)
    w["satrow"] = np.concatenate(
        [np.full(H, -KSAT, f16), np.full(H, KSAT, f16)])[None, :]
    return w


# ---------------------------------------------------------------------------
# device program
# ---------------------------------------------------------------------------

def _build():
    nc = bass.Bass("TRN2", target_bir_lowering=False, debug=False,
                   num_devices=NCORES)

    def di(name, shape, dtype=dt.float32):
        return nc.dram_tensor(name, shape, dtype, kind="ExternalInput")

    emb_d = di("emb", [VOCAB + 1, E])
    identf_d = di("identf", [P, P])
    identh_d = di("identh", [P, P], dt.float16)
    satrow_d = di("satrow", [1, 2 * H], dt.float16)
    idx_d = di("idx", [T * BC], dt.int32)
    minv_d = di("minv", [P, T, 2, BC], dt.uint8)
    minvmm_d = di("minvmm", [1, T, 2, BC], dt.float16)
    fcw_d = di("fcw", [2 * T, NCLS])
    fcb_d = di("fcb_rep", [BC, NCLS])
    wdram = {}
    for l in (0, 1):
        for d in ("f", "b"):
            wdram[f"wh{l}{d}"] = di(f"wh{l}{d}", [H, 4 * H], dt.float16)
            wdram[f"bcol{l}{d}"] = di(f"bcol{l}{d}", [P, 4])
            if l == 0:
                wdram[f"wx0{d}"] = di(f"wx0{d}", [E, 4 * H], dt.float16)
            else:
                wdram[f"wx1{d}f"] = di(f"wx1{d}f", [H, 4 * H], dt.float16)
                wdram[f"wx1{d}b"] = di(f"wx1{d}b", [H, 4 * H], dt.float16)

    out_d = nc.dram_tensor("out", [BC, NCLS], dt.float32, kind="ExternalOutput")
    feat_dram = nc.dram_tensor("feat", [2, T, BC], dt.float32)
    dbg = {}
    if DBG:
        for nm, sh, dty in [
            ("dxp0", [P, T, 2, 4, BC], dt.float16),
            ("dy0f", [P, T + 1, BC], dt.float16),
            ("dy0b", [P, T + 1, BC], dt.float16),
            ("dy1f", [P, T + 1, BC], dt.float16),
            ("dy1b", [P, T + 1, BC], dt.float16),
            ("dfmx", [P, 32], dt.float32),
            ("dfsum", [P, 32], dt.float32),
        ]:
            dbg[nm] = nc.dram_tensor(nm, sh, dty, kind="ExternalOutput")

    NTOK = T * BC            # 4096 tokens per core
    NCH = NTOK // P          # 32 gather/pool chunks
    NXC = 8                  # xp matmul chunks
    TCH = T // NXC           # 64 timesteps per xp chunk

    with tile.TileContext(nc) as tc:
        with (
            tc.tile_pool(name="const", bufs=1) as cpool,
            tc.tile_pool(name="big", bufs=1) as bigpool,
            tc.tile_pool(name="psz", bufs=4, space="PSUM") as psz,
        ):
            # ---- constant loads
            identf = cpool.tile([P, P], dt.float32, tag="identf")
            nc.sync.dma_start(out=identf[:], in_=identf_d[:])
            identh = cpool.tile([P, P], dt.float16, tag="identh")
            nc.sync.dma_start(out=identh[:], in_=identh_d[:])
            satrow = cpool.tile([1, 2 * H], dt.float16, tag="satrow")
            nc.sync.dma_start(out=satrow[:], in_=satrow_d[:])
            idx_t = cpool.tile([P, NCH], dt.int32, tag="idx")
            nc.sync.dma_start(
                out=idx_t[:], in_=idx_d.rearrange("(c p) -> p c", p=P))
            minv = cpool.tile([P, T, 2, BC], dt.uint8, tag="minv", name="minv")
            nc.sync.dma_start(out=minv[:], in_=minv_d[:])
            minvmm = cpool.tile([1, T, 2, BC], dt.float16, tag="minvmm",
                                name="minvmm")
            nc.sync.dma_start(out=minvmm[:], in_=minvmm_d[:])
            wsb = {}
            for k, dr in wdram.items():
                sh = list(dr.shape)
                wt = cpool.tile(sh, dr.dtype, tag=k, name=k)
                nc.sync.dma_start(out=wt[:], in_=dr[:])
                wsb[k] = wt
            fcw_t = cpool.tile([P, 2 * T // P, NCLS], dt.float32, tag="fcw")
            nc.sync.dma_start(
                out=fcw_t[:], in_=fcw_d.rearrange("(q p) c -> p q c", p=P))
            fcb_t = cpool.tile([BC, NCLS], dt.float32, tag="fcb")
            nc.sync.dma_start(out=fcb_t[:], in_=fcb_d[:])

            # big SBUF-resident tensors
            xp = bigpool.tile([P, T, 2, 4, BC], dt.float16, tag="xp", name="xp")
            y0 = {d: bigpool.tile([P, T + 1, BC], dt.float16,
                                  tag=f"y0{d}", name=f"y0{d}") for d in "fb"}
            y1 = {d: bigpool.tile([P, T + 1, BC], dt.float16,
                                  tag=f"y1{d}", name=f"y1{d}") for d in "fb"}
            g128 = bigpool.tile([P, T, BC], dt.float16, tag="g128", name="g128")

            # ---- embedding gather -> fp16 token matrix (h on partitions)
            g128f = g128[:].rearrange("p t b -> p (t b)")
            with tc.tile_pool(name="gph", bufs=3) as gpool, \
                 tc.tile_pool(name="psg", bufs=2, space="PSUM") as psg:
                for c in range(NCH):
                    gr = gpool.tile([P, E], dt.float32, tag="gr")
                    nc.gpsimd.indirect_dma_start(
                        out=gr[:], out_offset=None, in_=emb_d[:],
                        in_offset=bass.IndirectOffsetOnAxis(
                            ap=idx_t[:, c:c + 1], axis=0),
                    )
                    pt = psg.tile([P, P], dt.float32, tag="psg")
                    nc.tensor.transpose(out=pt[:], in_=gr[:], identity=identf[:])
                    nc.vector.tensor_copy(
                        out=g128f[:, c * P:(c + 1) * P], in_=pt[:])

            def xp_chunks(l, srcs):
                """Compute xp for layer l.

                srcs[d] = list of (lhsT_tile_key, rhs_view) accumulated per
                gate; bias added during PSUM->SBUF evac; +-K saturation for
                masked steps enters via a K=1 matmul against (1-mask).
                """
                with tc.tile_pool(name=f"psx{l}", bufs=3, space="PSUM") as psx:
                    for di_, d in enumerate("fb"):
                        bcol = wsb[f"bcol{l}{d}"]
                        for n in range(NXC):
                            t0, t1 = n * TCH, (n + 1) * TCH
                            for g in range(4):
                                px = psx.tile([P, TCH, BC], dt.float32,
                                              tag="px")
                                terms = srcs[d]
                                sat = g in (0, 1)
                                for ti_, (wkey, view) in enumerate(terms):
                                    last = (ti_ == len(terms) - 1) and not sat
                                    nc.tensor.matmul(
                                        out=px[:],
                                        lhsT=wsb[wkey][:, g * H:(g + 1) * H],
                                        rhs=view[:, t0:t1, :],
                                        start=(ti_ == 0), stop=last)
                                if sat:
                                    nc.tensor.matmul(
                                        out=px[:],
                                        lhsT=satrow[:, g * H:(g + 1) * H],
                                        rhs=minvmm[:, t0:t1, di_, :],
                                        start=False, stop=True)
                                nc.vector.tensor_scalar(
                                    out=xp[:, t0:t1, di_, g, :], in0=px[:],
                                    scalar1=bcol[:, g:g + 1], scalar2=None,
                                    op0=ALU.add)

            # ---- layer-0 xp from embeddings
            g3 = g128[:]  # [P, T, BC] view
            xp_chunks(0, {
                "f": [(f"wx0f", g3)],
                "b": [(f"wx0b", g3[:, ::-1, :])],
            })

            # ---- scan machinery
            SB = {d: cpool.tile([P, 9, BC], dt.float32, tag=f"SB{d}",
                                name=f"SB{d}")
                  for d in "fb"}

            def scan_layer(l, y):
                for di_, d in enumerate("fb"):
                    nc.vector.memset(SB[d][:, 4, :], 0.0)   # C'
                    nc.vector.memset(y[d][:, 0, :], 0.0)    # H'
                wh = {d: wsb[f"wh{l}{d}"] for d in "fb"}
                if True:  # fully static unroll over time
                    for tj in range(T):
                        tj1 = tj + 1
                        for di_, d in enumerate("fb"):
                            S = SB[d]
                            zp = psz.tile([P, 4, BC], dt.float32, tag="zp")
                            nc.tensor.matmul(
                                out=zp[:], lhsT=identh[:],
                                rhs=xp[:, tj, di_, :, :],
                                start=True, stop=False, skip_group_check=True)
                            for g in range(4):
                                nc.tensor.matmul(
                                    out=zp[:, g, :],
                                    lhsT=wh[d][:, g * H:(g + 1) * H],
                                    rhs=y[d][:, tj, :],
                                    start=False, stop=True,
                                    skip_group_check=True)
                            nc.scalar.activation(
                                out=S[:, 0:4, :], in_=zp[:], func=AF.Tanh,
                                scale=0.5)
                            # wt = (ti+1)*tg ; pt = (tf+1)*C'
                            nc.vector.scalar_tensor_tensor(
                                out=S[:, 5:7, :], in0=S[:, 0:2, :], scalar=1.0,
                                in1=S[:, 2:5:2, :], op0=ALU.add, op1=ALU.mult)
                            # C' = 0.5*pt + wt
                            nc.vector.scalar_tensor_tensor(
                                out=S[:, 4, :], in0=S[:, 6, :], scalar=0.5,
                                in1=S[:, 5, :], op0=ALU.mult, op1=ALU.add)
                            # tct = tanh(0.5*C')
                            nc.scalar.activation(
                                out=S[:, 7, :], in_=S[:, 4, :], func=AF.Tanh,
                                scale=0.5)
                            # H' = (to+1)*tct -> y[t+1]
                            nc.vector.scalar_tensor_tensor(
                                out=y[d][:, tj1, :], in0=S[:, 3, :],
                                scalar=1.0, in1=S[:, 7, :], op0=ALU.add,
                                op1=ALU.mult)
                            # masked steps carry previous H'
                            nc.vector.copy_predicated(
                                out=y[d][:, tj1, :],
                                mask=minv[:, tj, di_, :],
                                data=y[d][:, tj, :])

            if DBG:
                nc.sync.dma_start(out=dbg["dxp0"][:], in_=xp[:])
            scan_layer(0, y0)
            if DBG:
                nc.sync.dma_start(out=dbg["dy0f"][:], in_=y0["f"][:])
                nc.sync.dma_start(out=dbg["dy0b"][:], in_=y0["b"][:])

            # ---- layer-1 xp from SBUF y0
            yf = y0["f"][:, 1:T + 1, :]
            yb = y0["b"][:, 1:T + 1, :]
            yfr = yf[:, ::-1, :]
            ybr = yb[:, ::-1, :]
            xp_chunks(1, {
                "f": [("wx1ff", yf), ("wx1fb", ybr)],
                "b": [("wx1bf", yfr), ("wx1bb", yb)],
            })

            scan_layer(1, y1)
            if DBG:
                nc.sync.dma_start(out=dbg["dy1f"][:], in_=y1["f"][:])
                nc.sync.dma_start(out=dbg["dy1b"][:], in_=y1["b"][:])

            # ---- pooling over the 256 concat features per token
            fmx = cpool.tile([P, NCH], dt.float32, tag="fmx")
            fsum = cpool.tile([P, NCH], dt.float32, tag="fsum")
            yb_nat = y1["b"][:, 1:T + 1, :][:, ::-1, :]  # natural time
            with tc.tile_pool(name="psp", bufs=2, space="PSUM") as psp, \
                 tc.tile_pool(name="ystg", bufs=3) as ystgp:
                for c in range(NCH):
                    # stage to fp32 SBUF (the copy also un-reverses the
                    # scan-step-major b direction), transpose fp32, reduce.
                    ystg = ystgp.tile([P, 2, 16, BC], dt.float32, tag="ystg")
                    pt = psp.tile([P, 2, P], dt.float32, tag="pt")
                    srcs = (y1["f"][:, 1 + 16 * c:1 + 16 * (c + 1), :],
                            yb_nat[:, 16 * c:16 * (c + 1), :])
                    for di_ in range(2):
                        nc.gpsimd.tensor_copy(
                            out=ystg[:, di_, :, :], in_=srcs[di_])
                        nc.tensor.transpose(
                            out=pt[:, di_, :],
                            in_=ystg[:, di_, :, :],
                            identity=identf[:])
                    nc.vector.tensor_reduce(
                        out=fmx[:, c:c + 1], in_=pt[:],
                        axis=mybir.AxisListType.XYZW, op=ALU.max)
                    nc.vector.tensor_reduce(
                        out=fsum[:, c:c + 1], in_=pt[:],
                        axis=mybir.AxisListType.XYZW, op=ALU.add)

            if DBG:
                nc.sync.dma_start(out=dbg["dfmx"][:], in_=fmx[:])
                nc.sync.dma_start(out=dbg["dfsum"][:], in_=fsum[:])

            # ---- FC head via small DRAM bounce (transposed feat layout)
            with tc.tile_pool(name="ep", bufs=3) as epool, \
                 tc.tile_pool(name="psf", bufs=1, space="PSUM") as psf:
                featv = feat_dram  # [2, T, BC]
                nc.sync.dma_start(
                    out=featv[0].rearrange("(c q) b -> (q b) c", q=16),
                    in_=fmx[:])
                nc.sync.dma_start(
                    out=featv[1].rearrange("(c q) b -> (q b) c", q=16),
                    in_=fsum[:])
                pfc = psf.tile([BC, NCLS], dt.float32, tag="pfc")
                NQ = 2 * T // P
                for q in range(NQ):
                    lq = epool.tile([P, BC], dt.float32, tag="lq")
                    pool_i, tq = divmod(q * P, T)
                    nc.sync.dma_start(
                        out=lq[:], in_=feat_dram[pool_i, tq:tq + P, :])
                    nc.tensor.matmul(
                        out=pfc[:], lhsT=lq[:], rhs=fcw_t[:, q, :],
                        start=(q == 0), stop=(q == NQ - 1))
                ob = epool.tile([BC, NCLS], dt.float32, tag="ob")
                nc.vector.tensor_tensor(
                    out=ob[:], in0=pfc[:], in1=fcb_t[:], op=ALU.add)
                nc.vector.tensor_scalar(
                    out=ob[:], in0=ob[:], scalar1=0.0, scalar2=None,
                    op0=ALU.max)
                nc.sync.dma_start(out=out_d[:], in_=ob[:])

    split_multi_waits(nc)
    return nc


_cached_nc = None


def _get_nc():
    global _cached_nc
    if _cached_nc is None:
        _install_hook()
        _cached_nc = _build()
    return _cached_nc


def _in_maps(inputs):
    w = _fold_weights(inputs)
    x = np.asarray(inputs["x"]).astype(np.int32)  # [64, 512]
    shared = {
        "emb": w["emb"], "identf": w["identf"], "identh": w["identh"],
        "satrow": w["satrow"], "fcw": w["fcw"], "fcb_rep": w["fcb_rep"],
    }
    for l in (0, 1):
        for d in ("f", "b"):
            shared[f"wh{l}{d}"] = w[f"wh{l}{d}"]
            shared[f"bcol{l}{d}"] = w[f"bcol{l}{d}"]
            if l == 0:
                shared[f"wx0{d}"] = w[f"wx0{d}"]
            else:
                shared[f"wx1{d}f"] = w[f"wx1{d}f"]
                shared[f"wx1{d}b"] = w[f"wx1{d}b"]
    maps = []
    for c in range(NCORES):
        xc = x[c * BC:(c + 1) * BC]            # [BC, T]
        idx = np.ascontiguousarray(xc.T).reshape(-1).astype(np.int32)
        m = (xc != 0).T                        # [T, BC] bool
        minv_f = (~m).astype(np.uint8)         # forward-time inverted
        minv_b = minv_f[::-1]                  # scan-step s <-> t = T-1-s
        mi = np.stack([minv_f, minv_b], axis=1)  # [T, 2, BC]
        minv = np.broadcast_to(mi[None], (P, T, 2, BC))
        maps.append(dict(
            shared, idx=idx,
            minv=np.ascontiguousarray(minv),
            minvmm=np.ascontiguousarray(mi[None]).astype(np.float16),
        ))
    return maps


def _run(inputs, trace=False):
    from concourse.bass_utils import run_bass_kernel_spmd
    nc = _get_nc()
    maps = _in_maps(inputs)
    res = run_bass_kernel_spmd(nc, maps, list(range(NCORES)), trace=trace)
    out = np.concatenate([res.results[c]["out"] for c in range(NCORES)], axis=0)
    return out.astype(np.float32), res


def kernel(**inputs):
    out, _ = _run(inputs, trace=False)
    return out


def run_traced(inputs):
    out, res = _run(inputs, trace=True)
    return out, res


# revision 12
# speedup vs baseline: 5.1592x; 1.0043x over previous
"""Bass/TRN2 kernel for nn_BiRNNLayers: 2-layer BiLSTM (B=64, T=512, H=128,
vocab 50000) with masked Keras-style scan, feature pooling and FC head.

v2 strategy (8 NeuronCores, data-parallel over batch, 8 rows/core):
- fp16 weights/activations for the recurrent path (fast LDWEIGHTS+MATMUL,
  2x DVE), fp32 cell state and gate math.
- One tanh table for everything: every activation is Tanh(scale=0.5); the
  g-gate columns are pre-scaled by 2 so tanh(z_g) comes out right. No
  ACT_TABLE_LOAD thrash.
- xp (input projections incl. bias and +-K mask saturation) precomputed
  into SBUF fp16; per step it is preloaded into PSUM with an identity
  matmul and the 4 recurrent gate matmuls accumulate on top.
- State kept as H'=2h (fp16, in the y tile) and C'=2c (fp32). Cell math:
  tanh gates -> fused (t+1)*x ops on DVE, h' on GpSimd, masked-carry
  copy_predicated on DVE. y stays SBUF-resident; no per-step DMA.
- For_i with UNROLL=32 to amortize the tile-loop barrier.
- Layer-1 xp read straight from SBUF y0 (time-reversed views for the
  opposite direction); pooling via PE transposes of SBUF y1 + DVE
  reduces; FC through a small DRAM bounce for the transposed layout.
"""
import os
import numpy as np

import concourse.bass as bass
import concourse.mybir as mybir
import concourse.tile as tile
import bass_rust

P = 128
T = 512
H = 128
E = 128
B_FULL = 64
NCORES = 8
BC = B_FULL // NCORES  # batch rows per core
VOCAB = 50000
NCLS = 10
KSAT = 40.0            # pre-activation saturation offset for masked steps
UNROLL = 32
DBG = os.environ.get("KDBG", "0") == "1"

AF = mybir.ActivationFunctionType
ALU = mybir.AluOpType
dt = mybir.dt

_hook_installed = False


def _install_hook():
    """Surface compile-hook tracebacks (PJRT swallows them otherwise)."""
    global _hook_installed
    if _hook_installed:
        return
    _hook_installed = True
    import traceback
    import concourse.bass2jax as bass2jax
    import libneuronxla

    orig = bass2jax.neuronx_cc_hook

    def dbg_hook(*a, **k):
        try:
            return orig(*a, **k)
        except BaseException:
            traceback.print_exc()
            raise

    bass2jax.neuronx_cc_hook = dbg_hook
    if not hasattr(libneuronxla, "orig_neuronx_cc"):
        libneuronxla.orig_neuronx_cc = libneuronxla.neuronx_cc
    libneuronxla.neuronx_cc = dbg_hook


def split_multi_waits(nc):
    """This container's walrus encodes at most one sem wait per instruction;
    hoist extra waits onto preceding same-engine NoOps."""
    for fn in nc.m.functions:
        for bb in fn.blocks:
            out = []
            changed = False
            for inst in bb.instructions:
                si = inst.sync_info
                waits = list(si.on_wait) if si is not None and si.on_wait else []
                if len(waits) > 1:
                    changed = True
                    for k, w in enumerate(waits[:-1]):
                        nop = mybir.InstNoOp(name=f"{inst.name}-sw{k}")
                        nop.engine = inst.engine
                        nop.sync_info = bass_rust.SyncInfo(on_wait=[w], on_update=[])
                        out.append(nop)
                    inst.sync_info = bass_rust.SyncInfo(
                        on_wait=[waits[-1]], on_update=list(si.on_update)
                    )
                out.append(inst)
            if changed:
                bb.instructions = out


# ---------------------------------------------------------------------------
# host-side weight folding
# ---------------------------------------------------------------------------

def _fold_weights(inputs):
    f32, f16 = np.float32, np.float16
    # every activation is tanh(0.5*z'): i,f,o columns raw (sigmoid =
    # (tanh(z/2)+1)/2), g columns pre-scaled by 2 so tanh(0.5*2*z)=tanh(z).
    cs = np.concatenate([
        np.ones(H), np.ones(H), np.full(H, 2.0), np.ones(H)
    ]).astype(f32)

    w = {}
    for l in (0, 1):
        for d in ("f", "b"):
            Wx = np.asarray(inputs[f"Wx_{d}{l}"], f32)
            Wh = np.asarray(inputs[f"Wh_{d}{l}"], f32)
            b = np.asarray(inputs[f"b_{d}{l}"], f32)
            # recurrent input is H'=2h -> fold 0.5 into Wh
            w[f"wh{l}{d}"] = ((Wh * 0.5) * cs).astype(f16)
            be = (b * cs).astype(f32)
            w[f"bcol{l}{d}"] = np.ascontiguousarray(
                be.reshape(4, H).T)  # [128, 4] f32
            if l == 0:
                w[f"wx0{d}"] = (Wx * cs).astype(f16)
            else:
                w[f"wx1{d}f"] = ((Wx[0:H] * 0.5) * cs).astype(f16)
                w[f"wx1{d}b"] = ((Wx[H:2 * H] * 0.5) * cs).astype(f16)

    w["emb"] = np.asarray(inputs["emb"], f32)

    fcw = np.asarray(inputs["fc_W"], f32).copy()  # [2T, 10]
    fcw[:T] *= 0.5          # mx rows: feat carries 2*mx
    fcw[T:] *= 1.0 / 512.0  # av rows: feat carries sum(2h) over 256 feats
    w["fcw"] = fcw.astype(f32)
    w["fcb_rep"] = np.tile(np.asarray(inputs["fc_b"], f32)[None, :], (BC, 1))
    w["identf"] = np.eye(P, dtype=f32)
    w["identh"] = np.eye(P, dtype=f16)
    # saturation rows: [1, 2H]: first H = -K (i gate), second H = +K (f gate)
    w["satrow"] = np.concatenate(
        [np.full(H, -KSAT, f16), np.full(H, KSAT, f16)])[None, :]
    return w


# ---------------------------------------------------------------------------
# device program
# ---------------------------------------------------------------------------

def _build():
    nc = bass.Bass("TRN2", target_bir_lowering=False, debug=False,
                   num_devices=NCORES)

    def di(name, shape, dtype=dt.float32):
        return nc.dram_tensor(name, shape, dtype, kind="ExternalInput")

    emb_d = di("emb", [VOCAB + 1, E])
    identf_d = di("identf", [P, P])
    identh_d = di("identh", [P, P], dt.float16)
    satrow_d = di("satrow", [1, 2 * H], dt.float16)
    idx_d = di("idx", [T * BC], dt.int32)
    minv_d = di("minv", [P, T, 2, BC], dt.uint8)
    minvmm_d = di("minvmm", [1, T, 2, BC], dt.float16)
    fcw_d = di("fcw", [2 * T, NCLS])
    fcb_d = di("fcb_rep", [BC, NCLS])
    wdram = {}
    for l in (0, 1):
        for d in ("f", "b"):
            wdram[f"wh{l}{d}"] = di(f"wh{l}{d}", [H, 4 * H], dt.float16)
            wdram[f"bcol{l}{d}"] = di(f"bcol{l}{d}", [P, 4])
            if l == 0:
                wdram[f"wx0{d}"] = di(f"wx0{d}", [E, 4 * H], dt.float16)
            else:
                wdram[f"wx1{d}f"] = di(f"wx1{d}f", [H, 4 * H], dt.float16)
                wdram[f"wx1{d}b"] = di(f"wx1{d}b", [H, 4 * H], dt.float16)

    out_d = nc.dram_tensor("out", [BC, NCLS], dt.float32, kind="ExternalOutput")
    feat_dram = nc.dram_tensor("feat", [2, T, BC], dt.float32)
    dbg = {}
    if DBG:
        for nm, sh, dty in [
            ("dxp0", [P, T, 2, 4, BC], dt.float16),
            ("dy0f", [P, T + 1, BC], dt.float16),
            ("dy0b", [P, T + 1, BC], dt.float16),
            ("dy1f", [P, T + 1, BC], dt.float16),
            ("dy1b", [P, T + 1, BC], dt.float16),
            ("dfmx", [P, 32], dt.float32),
            ("dfsum", [P, 32], dt.float32),
        ]:
            dbg[nm] = nc.dram_tensor(nm, sh, dty, kind="ExternalOutput")

    NTOK = T * BC            # 4096 tokens per core
    NCH = NTOK // P          # 32 gather/pool chunks
    NXC = 8                  # xp matmul chunks
    TCH = T // NXC           # 64 timesteps per xp chunk

    with tile.TileContext(nc) as tc:
        with (
            tc.tile_pool(name="const", bufs=1) as cpool,
            tc.tile_pool(name="big", bufs=1) as bigpool,
            tc.tile_pool(name="psz", bufs=4, space="PSUM") as psz,
        ):
            # ---- constant loads
            identf = cpool.tile([P, P], dt.float32, tag="identf")
            nc.sync.dma_start(out=identf[:], in_=identf_d[:])
            identh = cpool.tile([P, P], dt.float16, tag="identh")
            nc.sync.dma_start(out=identh[:], in_=identh_d[:])
            satrow = cpool.tile([1, 2 * H], dt.float16, tag="satrow")
            nc.sync.dma_start(out=satrow[:], in_=satrow_d[:])
            idx_t = cpool.tile([P, NCH], dt.int32, tag="idx")
            nc.sync.dma_start(
                out=idx_t[:], in_=idx_d.rearrange("(c p) -> p c", p=P))
            minv = cpool.tile([P, T, 2, BC], dt.uint8, tag="minv", name="minv")
            nc.sync.dma_start(out=minv[:], in_=minv_d[:])
            minvmm = cpool.tile([1, T, 2, BC], dt.float16, tag="minvmm",
                                name="minvmm")
            nc.sync.dma_start(out=minvmm[:], in_=minvmm_d[:])
            wsb = {}
            for k, dr in wdram.items():
                sh = list(dr.shape)
                wt = cpool.tile(sh, dr.dtype, tag=k, name=k)
                nc.sync.dma_start(out=wt[:], in_=dr[:])
                wsb[k] = wt
            fcw_t = cpool.tile([P, 2 * T // P, NCLS], dt.float32, tag="fcw")
            nc.sync.dma_start(
                out=fcw_t[:], in_=fcw_d.rearrange("(q p) c -> p q c", p=P))
            fcb_t = cpool.tile([BC, NCLS], dt.float32, tag="fcb")
            nc.sync.dma_start(out=fcb_t[:], in_=fcb_d[:])

            # big SBUF-resident tensors
            xp = bigpool.tile([P, T, 2, 4, BC], dt.float16, tag="xp", name="xp")
            y0 = {d: bigpool.tile([P, T + 1, BC], dt.float16,
                                  tag=f"y0{d}", name=f"y0{d}") for d in "fb"}
            y1 = {d: bigpool.tile([P, T + 1, BC], dt.float16,
                                  tag=f"y1{d}", name=f"y1{d}") for d in "fb"}
            g128 = bigpool.tile([P, T, BC], dt.float16, tag="g128", name="g128")

            # ---- embedding gather -> fp16 token matrix (h on partitions)
            g128f = g128[:].rearrange("p t b -> p (t b)")
            with tc.tile_pool(name="gph", bufs=3) as gpool, \
                 tc.tile_pool(name="psg", bufs=2, space="PSUM") as psg:
                for c in range(NCH):
                    gr = gpool.tile([P, E], dt.float32, tag="gr")
                    nc.gpsimd.indirect_dma_start(
                        out=gr[:], out_offset=None, in_=emb_d[:],
                        in_offset=bass.IndirectOffsetOnAxis(
                            ap=idx_t[:, c:c + 1], axis=0),
                    )
                    pt = psg.tile([P, P], dt.float32, tag="psg")
                    nc.tensor.transpose(out=pt[:], in_=gr[:], identity=identf[:])
                    nc.vector.tensor_copy(
                        out=g128f[:, c * P:(c + 1) * P], in_=pt[:])

            def xp_chunks(l, srcs):
                """Compute xp for layer l.

                srcs[d] = list of (lhsT_tile_key, rhs_view) accumulated per
                gate; bias added during PSUM->SBUF evac; +-K saturation for
                masked steps enters via a K=1 matmul against (1-mask).
                """
                with tc.tile_pool(name=f"psx{l}", bufs=3, space="PSUM") as psx:
                    for di_, d in enumerate("fb"):
                        bcol = wsb[f"bcol{l}{d}"]
                        for n in range(NXC):
                            t0, t1 = n * TCH, (n + 1) * TCH
                            for g in range(4):
                                px = psx.tile([P, TCH, BC], dt.float32,
                                              tag="px")
                                terms = srcs[d]
                                sat = g in (0, 1)
                                for ti_, (wkey, view) in enumerate(terms):
                                    last = (ti_ == len(terms) - 1) and not sat
                                    nc.tensor.matmul(
                                        out=px[:],
                                        lhsT=wsb[wkey][:, g * H:(g + 1) * H],
                                        rhs=view[:, t0:t1, :],
                                        start=(ti_ == 0), stop=last)
                                if sat:
                                    nc.tensor.matmul(
                                        out=px[:],
                                        lhsT=satrow[:, g * H:(g + 1) * H],
                                        rhs=minvmm[:, t0:t1, di_, :],
                                        start=False, stop=True)
                                nc.vector.tensor_scalar(
                                    out=xp[:, t0:t1, di_, g, :], in0=px[:],
                                    scalar1=bcol[:, g:g + 1], scalar2=None,
                                    op0=ALU.add)

            # ---- layer-0 xp from embeddings
            g3 = g128[:]  # [P, T, BC] view
            xp_chunks(0, {
                "f": [(f"wx0f", g3)],
                "b": [(f"wx0b", g3[:, ::-1, :])],
            })

            # ---- scan machinery
            SB = {d: cpool.tile([P, 9, BC], dt.float32, tag=f"SB{d}",
                                name=f"SB{d}")
                  for d in "fb"}

            def scan_layer(l, y):
                for di_, d in enumerate("fb"):
                    nc.vector.memset(SB[d][:, 4, :], 0.0)   # C'
                    nc.vector.memset(y[d][:, 0, :], 0.0)    # H'
                wh = {d: wsb[f"wh{l}{d}"] for d in "fb"}
                if True:  # fully static unroll, dir-interleaved emission
                    for tj in range(T):
                        tj1 = tj + 1
                        zps = {}
                        for di_, d in enumerate("fb"):
                            zp = psz.tile([P, 4, BC], dt.float32, tag="zp")
                            nc.tensor.matmul(
                                out=zp[:], lhsT=identh[:],
                                rhs=xp[:, tj, di_, :, :],
                                start=True, stop=False, skip_group_check=True)
                            for g in range(4):
                                nc.tensor.matmul(
                                    out=zp[:, g, :],
                                    lhsT=wh[d][:, g * H:(g + 1) * H],
                                    rhs=y[d][:, tj, :],
                                    start=False, stop=True,
                                    skip_group_check=True)
                            zps[d] = zp
                        for d in "fb":
                            nc.scalar.activation(
                                out=SB[d][:, 0:4, :], in_=zps[d][:],
                                func=AF.Tanh, scale=0.5)
                        for d in "fb":
                            S = SB[d]
                            nc.vector.scalar_tensor_tensor(
                                out=S[:, 5:7, :], in0=S[:, 0:2, :], scalar=1.0,
                                in1=S[:, 2:5:2, :], op0=ALU.add, op1=ALU.mult)
                        for d in "fb":
                            S = SB[d]
                            nc.vector.scalar_tensor_tensor(
                                out=S[:, 4, :], in0=S[:, 6, :], scalar=0.5,
                                in1=S[:, 5, :], op0=ALU.mult, op1=ALU.add)
                        for d in "fb":
                            S = SB[d]
                            nc.scalar.activation(
                                out=S[:, 7, :], in_=S[:, 4, :], func=AF.Tanh,
                                scale=0.5)
                        for di_, d in enumerate("fb"):
                            S = SB[d]
                            nc.vector.scalar_tensor_tensor(
                                out=y[d][:, tj1, :], in0=S[:, 3, :],
                                scalar=1.0, in1=S[:, 7, :], op0=ALU.add,
                                op1=ALU.mult)
                            nc.vector.copy_predicated(
                                out=y[d][:, tj1, :],
                                mask=minv[:, tj, di_, :],
                                data=y[d][:, tj, :])

            scan_layer(0, y0)
            if DBG:
                nc.sync.dma_start(out=dbg["dy0f"][:], in_=y0["f"][:])
                nc.sync.dma_start(out=dbg["dy0b"][:], in_=y0["b"][:])

            # ---- layer-1 xp from SBUF y0
            yf = y0["f"][:, 1:T + 1, :]
            yb = y0["b"][:, 1:T + 1, :]
            yfr = yf[:, ::-1, :]
            ybr = yb[:, ::-1, :]
            xp_chunks(1, {
                "f": [("wx1ff", yf), ("wx1fb", ybr)],
                "b": [("wx1bf", yfr), ("wx1bb", yb)],
            })

            scan_layer(1, y1)
            if DBG:
                nc.sync.dma_start(out=dbg["dy1f"][:], in_=y1["f"][:])
                nc.sync.dma_start(out=dbg["dy1b"][:], in_=y1["b"][:])

            # ---- pooling over the 256 concat features per token
            fmx = cpool.tile([P, NCH], dt.float32, tag="fmx")
            fsum = cpool.tile([P, NCH], dt.float32, tag="fsum")
            yb_nat = y1["b"][:, 1:T + 1, :][:, ::-1, :]  # natural time
            with tc.tile_pool(name="psp", bufs=2, space="PSUM") as psp, \
                 tc.tile_pool(name="ystg", bufs=3) as ystgp:
                for c in range(NCH):
                    # stage to fp32 SBUF (the copy also un-reverses the
                    # scan-step-major b direction), transpose fp32, reduce.
                    ystg = ystgp.tile([P, 2, 16, BC], dt.float32, tag="ystg")
                    pt = psp.tile([P, 2, P], dt.float32, tag="pt")
                    srcs = (y1["f"][:, 1 + 16 * c:1 + 16 * (c + 1), :],
                            yb_nat[:, 16 * c:16 * (c + 1), :])
                    for di_ in range(2):
                        nc.gpsimd.tensor_copy(
                            out=ystg[:, di_, :, :], in_=srcs[di_])
                        nc.tensor.transpose(
                            out=pt[:, di_, :],
                            in_=ystg[:, di_, :, :],
                            identity=identf[:])
                    nc.vector.tensor_reduce(
                        out=fmx[:, c:c + 1], in_=pt[:],
                        axis=mybir.AxisListType.XYZW, op=ALU.max)
                    nc.vector.tensor_reduce(
                        out=fsum[:, c:c + 1], in_=pt[:],
                        axis=mybir.AxisListType.XYZW, op=ALU.add)

            if DBG:
                nc.sync.dma_start(out=dbg["dfmx"][:], in_=fmx[:])
                nc.sync.dma_start(out=dbg["dfsum"][:], in_=fsum[:])

            # ---- FC head via small DRAM bounce (transposed feat layout)
            with tc.tile_pool(name="ep", bufs=3) as epool, \
                 tc.tile_pool(name="psf", bufs=1, space="PSUM") as psf:
                featv = feat_dram  # [2, T, BC]
                nc.sync.dma_start(
                    out=featv[0].rearrange("(c q) b -> (q b) c", q=16),
                    in_=fmx[:])
                nc.sync.dma_start(
                    out=featv[1].rearrange("(c q) b -> (q b) c", q=16),
                    in_=fsum[:])
                pfc = psf.tile([BC, NCLS], dt.float32, tag="pfc")
                NQ = 2 * T // P
                for q in range(NQ):
                    lq = epool.tile([P, BC], dt.float32, tag="lq")
                    pool_i, tq = divmod(q * P, T)
                    nc.sync.dma_start(
                        out=lq[:], in_=feat_dram[pool_i, tq:tq + P, :])
                    nc.tensor.matmul(
                        out=pfc[:], lhsT=lq[:], rhs=fcw_t[:, q, :],
                        start=(q == 0), stop=(q == NQ - 1))
                ob = epool.tile([BC, NCLS], dt.float32, tag="ob")
                nc.vector.tensor_tensor(
                    out=ob[:], in0=pfc[:], in1=fcb_t[:], op=ALU.add)
                nc.vector.tensor_scalar(
                    out=ob[:], in0=ob[:], scalar1=0.0, scalar2=None,
                    op0=ALU.max)
                nc.sync.dma_start(out=out_d[:], in_=ob[:])

    split_multi_waits(nc)
    return nc


_cached_nc = None


def _get_nc():
    global _cached_nc
    if _cached_nc is None:
        _install_hook()
        _cached_nc = _build()
    return _cached_nc


def _in_maps(inputs):
    w = _fold_weights(inputs)
    x = np.asarray(inputs["x"]).astype(np.int32)  # [64, 512]
    shared = {
        "emb": w["emb"], "identf": w["identf"], "identh": w["identh"],
        "satrow": w["satrow"], "fcw": w["fcw"], "fcb_rep": w["fcb_rep"],
    }
    for l in (0, 1):
        for d in ("f", "b"):
            shared[f"wh{l}{d}"] = w[f"wh{l}{d}"]
            shared[f"bcol{l}{d}"] = w[f"bcol{l}{d}"]
            if l == 0:
                shared[f"wx0{d}"] = w[f"wx0{d}"]
            else:
                shared[f"wx1{d}f"] = w[f"wx1{d}f"]
                shared[f"wx1{d}b"] = w[f"wx1{d}b"]
    maps = []
    for c in range(NCORES):
        xc = x[c * BC:(c + 1) * BC]            # [BC, T]
        idx = np.ascontiguousarray(xc.T).reshape(-1).astype(np.int32)
        m = (xc != 0).T                        # [T, BC] bool
        minv_f = (~m).astype(np.uint8)         # forward-time inverted
        minv_b = minv_f[::-1]                  # scan-step s <-> t = T-1-s
        mi = np.stack([minv_f, minv_b], axis=1)  # [T, 2, BC]
        minv = np.broadcast_to(mi[None], (P, T, 2, BC))
        maps.append(dict(
            shared, idx=idx,
            minv=np.ascontiguousarray(minv),
            minvmm=np.ascontiguousarray(mi[None]).astype(np.float16),
        ))
    return maps


def _run(inputs, trace=False):
    from concourse.bass_utils import run_bass_kernel_spmd
    nc = _get_nc()
    maps = _in_maps(inputs)
    res = run_bass_kernel_spmd(nc, maps, list(range(NCORES)), trace=trace)
    out = np.concatenate([res.results[c]["out"] for c in range(NCORES)], axis=0)
    return out.astype(np.float32), res


def kernel(**inputs):
    out, _ = _run(inputs, trace=False)
    return out


def run_traced(inputs):
    out, res = _run(inputs, trace=True)
    return out, res


# revision 13
# speedup vs baseline: 5.2018x; 1.0083x over previous
"""Bass/TRN2 kernel for nn_BiRNNLayers: 2-layer BiLSTM (B=64, T=512, H=128,
vocab 50000) with masked Keras-style scan, feature pooling and FC head.

v2 strategy (8 NeuronCores, data-parallel over batch, 8 rows/core):
- fp16 weights/activations for the recurrent path (fast LDWEIGHTS+MATMUL,
  2x DVE), fp32 cell state and gate math.
- One tanh table for everything: every activation is Tanh(scale=0.5); the
  g-gate columns are pre-scaled by 2 so tanh(z_g) comes out right. No
  ACT_TABLE_LOAD thrash.
- xp (input projections incl. bias and +-K mask saturation) precomputed
  into SBUF fp16; per step it is preloaded into PSUM with an identity
  matmul and the 4 recurrent gate matmuls accumulate on top.
- State kept as H'=2h (fp16, in the y tile) and C'=2c (fp32). Cell math:
  tanh gates -> fused (t+1)*x ops on DVE, h' on GpSimd, masked-carry
  copy_predicated on DVE. y stays SBUF-resident; no per-step DMA.
- For_i with UNROLL=32 to amortize the tile-loop barrier.
- Layer-1 xp read straight from SBUF y0 (time-reversed views for the
  opposite direction); pooling via PE transposes of SBUF y1 + DVE
  reduces; FC through a small DRAM bounce for the transposed layout.
"""
import os
import numpy as np

import concourse.bass as bass
import concourse.mybir as mybir
import concourse.tile as tile
import bass_rust

P = 128
T = 512
H = 128
E = 128
B_FULL = 64
NCORES = 8
BC = B_FULL // NCORES  # batch rows per core
VOCAB = 50000
NCLS = 10
KSAT = 40.0            # pre-activation saturation offset for masked steps
UNROLL = 32
DBG = os.environ.get("KDBG", "0") == "1"

AF = mybir.ActivationFunctionType
ALU = mybir.AluOpType
dt = mybir.dt

_hook_installed = False


def _install_hook():
    """Surface compile-hook tracebacks (PJRT swallows them otherwise)."""
    global _hook_installed
    if _hook_installed:
        return
    _hook_installed = True
    import traceback
    import concourse.bass2jax as bass2jax
    import libneuronxla

    orig = bass2jax.neuronx_cc_hook

    def dbg_hook(*a, **k):
        try:
            return orig(*a, **k)
        except BaseException:
            traceback.print_exc()
            raise

    bass2jax.neuronx_cc_hook = dbg_hook
    if not hasattr(libneuronxla, "orig_neuronx_cc"):
        libneuronxla.orig_neuronx_cc = libneuronxla.neuronx_cc
    libneuronxla.neuronx_cc = dbg_hook


def split_multi_waits(nc):
    """This container's walrus encodes at most one sem wait per instruction;
    hoist extra waits onto preceding same-engine NoOps."""
    for fn in nc.m.functions:
        for bb in fn.blocks:
            out = []
            changed = False
            for inst in bb.instructions:
                si = inst.sync_info
                waits = list(si.on_wait) if si is not None and si.on_wait else []
                if len(waits) > 1:
                    changed = True
                    for k, w in enumerate(waits[:-1]):
                        nop = mybir.InstNoOp(name=f"{inst.name}-sw{k}")
                        nop.engine = inst.engine
                        nop.sync_info = bass_rust.SyncInfo(on_wait=[w], on_update=[])
                        out.append(nop)
                    inst.sync_info = bass_rust.SyncInfo(
                        on_wait=[waits[-1]], on_update=list(si.on_update)
                    )
                out.append(inst)
            if changed:
                bb.instructions = out


# ---------------------------------------------------------------------------
# host-side weight folding
# ---------------------------------------------------------------------------

def _fold_weights(inputs):
    f32, f16 = np.float32, np.float16
    # every activation is tanh(0.5*z'): i,f,o columns raw (sigmoid =
    # (tanh(z/2)+1)/2), g columns pre-scaled by 2 so tanh(0.5*2*z)=tanh(z).
    cs = np.concatenate([
        np.ones(H), np.ones(H), np.full(H, 2.0), np.ones(H)
    ]).astype(f32)

    w = {}
    for l in (0, 1):
        for d in ("f", "b"):
            Wx = np.asarray(inputs[f"Wx_{d}{l}"], f32)
            Wh = np.asarray(inputs[f"Wh_{d}{l}"], f32)
            b = np.asarray(inputs[f"b_{d}{l}"], f32)
            # recurrent input is H'=2h -> fold 0.5 into Wh
            w[f"wh{l}{d}"] = ((Wh * 0.5) * cs).astype(f16)
            be = (b * cs).astype(f32)
            w[f"bcol{l}{d}"] = np.ascontiguousarray(
                be.reshape(4, H).T)  # [128, 4] f32
            if l == 0:
                w[f"wx0{d}"] = (Wx * cs).astype(f16)
            else:
                w[f"wx1{d}f"] = ((Wx[0:H] * 0.5) * cs).astype(f16)
                w[f"wx1{d}b"] = ((Wx[H:2 * H] * 0.5) * cs).astype(f16)

    w["emb"] = np.asarray(inputs["emb"], f32)

    fcw = np.asarray(inputs["fc_W"], f32).copy()  # [2T, 10]
    fcw[:T] *= 0.5          # mx rows: feat carries 2*mx
    fcw[T:] *= 1.0 / 512.0  # av rows: feat carries sum(2h) over 256 feats
    w["fcw"] = fcw.astype(f32)
    w["fcb_rep"] = np.tile(np.asarray(inputs["fc_b"], f32)[None, :], (BC, 1))
    w["identf"] = np.eye(P, dtype=f32)
    w["identh"] = np.eye(P, dtype=f16)
    # saturation rows: [1, 2H]: first H = -K (i gate), second H = +K (f gate)
    w["satrow"] = np.concatenate(
        [np.full(H, -KSAT, f16), np.full(H, KSAT, f16)])[None, :]
    return w


# ---------------------------------------------------------------------------
# device program
# ---------------------------------------------------------------------------

def _build():
    nc = bass.Bass("TRN2", target_bir_lowering=False, debug=False,
                   num_devices=NCORES)

    def di(name, shape, dtype=dt.float32):
        return nc.dram_tensor(name, shape, dtype, kind="ExternalInput")

    emb_d = di("emb", [VOCAB + 1, E])
    identf_d = di("identf", [P, P])
    identh_d = di("identh", [P, P], dt.float16)
    satrow_d = di("satrow", [1, 2 * H], dt.float16)
    idx_d = di("idx", [T * BC], dt.int32)
    minv_d = di("minv", [P, T, 2, BC], dt.uint8)
    minvmm_d = di("minvmm", [1, T, 2, BC], dt.float16)
    fcw_d = di("fcw", [2 * T, NCLS])
    fcb_d = di("fcb_rep", [BC, NCLS])
    wdram = {}
    for l in (0, 1):
        for d in ("f", "b"):
            wdram[f"wh{l}{d}"] = di(f"wh{l}{d}", [H, 4 * H], dt.float16)
            wdram[f"bcol{l}{d}"] = di(f"bcol{l}{d}", [P, 4])
            if l == 0:
                wdram[f"wx0{d}"] = di(f"wx0{d}", [E, 4 * H], dt.float16)
            else:
                wdram[f"wx1{d}f"] = di(f"wx1{d}f", [H, 4 * H], dt.float16)
                wdram[f"wx1{d}b"] = di(f"wx1{d}b", [H, 4 * H], dt.float16)

    out_d = nc.dram_tensor("out", [BC, NCLS], dt.float32, kind="ExternalOutput")
    feat_dram = nc.dram_tensor("feat", [2, T, BC], dt.float32)
    dbg = {}
    if DBG:
        for nm, sh, dty in [
            ("dxp0", [P, T, 2, 4, BC], dt.float16),
            ("dy0f", [P, T + 1, BC], dt.float16),
            ("dy0b", [P, T + 1, BC], dt.float16),
            ("dy1f", [P, T + 1, BC], dt.float16),
            ("dy1b", [P, T + 1, BC], dt.float16),
            ("dfmx", [P, 32], dt.float32),
            ("dfsum", [P, 32], dt.float32),
        ]:
            dbg[nm] = nc.dram_tensor(nm, sh, dty, kind="ExternalOutput")

    NTOK = T * BC            # 4096 tokens per core
    NCH = NTOK // P          # 32 gather/pool chunks
    NXC = 8                  # xp matmul chunks
    TCH = T // NXC           # 64 timesteps per xp chunk

    with tile.TileContext(nc) as tc:
        with (
            tc.tile_pool(name="const", bufs=1) as cpool,
            tc.tile_pool(name="big", bufs=1) as bigpool,
            tc.tile_pool(name="psz", bufs=4, space="PSUM") as psz,
        ):
            # ---- constant loads
            identf = cpool.tile([P, P], dt.float32, tag="identf")
            nc.sync.dma_start(out=identf[:], in_=identf_d[:])
            identh = cpool.tile([P, P], dt.float16, tag="identh")
            nc.sync.dma_start(out=identh[:], in_=identh_d[:])
            satrow = cpool.tile([1, 2 * H], dt.float16, tag="satrow")
            nc.sync.dma_start(out=satrow[:], in_=satrow_d[:])
            idx_t = cpool.tile([P, NCH], dt.int32, tag="idx")
            nc.sync.dma_start(
                out=idx_t[:], in_=idx_d.rearrange("(c p) -> p c", p=P))
            minv = cpool.tile([P, T, 2, BC], dt.uint8, tag="minv", name="minv")
            nc.sync.dma_start(out=minv[:], in_=minv_d[:])
            minvmm = cpool.tile([1, T, 2, BC], dt.float16, tag="minvmm",
                                name="minvmm")
            nc.sync.dma_start(out=minvmm[:], in_=minvmm_d[:])
            wsb = {}
            for k, dr in wdram.items():
                sh = list(dr.shape)
                wt = cpool.tile(sh, dr.dtype, tag=k, name=k)
                nc.sync.dma_start(out=wt[:], in_=dr[:])
                wsb[k] = wt
            fcw_t = cpool.tile([P, 2 * T // P, NCLS], dt.float32, tag="fcw")
            nc.sync.dma_start(
                out=fcw_t[:], in_=fcw_d.rearrange("(q p) c -> p q c", p=P))
            fcb_t = cpool.tile([BC, NCLS], dt.float32, tag="fcb")
            nc.sync.dma_start(out=fcb_t[:], in_=fcb_d[:])

            # big SBUF-resident tensors
            xp = bigpool.tile([P, T, 2, 4, BC], dt.float16, tag="xp", name="xp")
            y0 = {d: bigpool.tile([P, T + 1, BC], dt.float16,
                                  tag=f"y0{d}", name=f"y0{d}") for d in "fb"}
            y1 = {d: bigpool.tile([P, T + 1, BC], dt.float16,
                                  tag=f"y1{d}", name=f"y1{d}") for d in "fb"}
            g128 = bigpool.tile([P, T, BC], dt.float16, tag="g128", name="g128")

            # ---- embedding gather -> fp16 token matrix (h on partitions)
            g128f = g128[:].rearrange("p t b -> p (t b)")
            with tc.tile_pool(name="gph", bufs=3) as gpool, \
                 tc.tile_pool(name="psg", bufs=2, space="PSUM") as psg:
                for c in range(NCH):
                    gr = gpool.tile([P, E], dt.float32, tag="gr")
                    nc.gpsimd.indirect_dma_start(
                        out=gr[:], out_offset=None, in_=emb_d[:],
                        in_offset=bass.IndirectOffsetOnAxis(
                            ap=idx_t[:, c:c + 1], axis=0),
                    )
                    pt = psg.tile([P, P], dt.float32, tag="psg")
                    nc.tensor.transpose(out=pt[:], in_=gr[:], identity=identf[:])
                    nc.vector.tensor_copy(
                        out=g128f[:, c * P:(c + 1) * P], in_=pt[:])

            def xp_chunk_n(l, srcs, psx, n):
                """Compute xp chunk n (both dirs) for layer l.

                srcs[d] = list of (lhsT_tile_key, rhs_view) accumulated per
                gate; bias added during PSUM->SBUF evac; +-K saturation for
                masked steps enters via a K=1 matmul against (1-mask).
                """
                if True:
                    for di_, d in enumerate("fb"):
                        bcol = wsb[f"bcol{l}{d}"]
                        if True:
                            t0, t1 = n * TCH, (n + 1) * TCH
                            for g in range(4):
                                px = psx.tile([P, TCH, BC], dt.float32,
                                              tag="px")
                                terms = srcs[d]
                                sat = g in (0, 1)
                                for ti_, (wkey, view) in enumerate(terms):
                                    last = (ti_ == len(terms) - 1) and not sat
                                    nc.tensor.matmul(
                                        out=px[:],
                                        lhsT=wsb[wkey][:, g * H:(g + 1) * H],
                                        rhs=view[:, t0:t1, :],
                                        start=(ti_ == 0), stop=last)
                                if sat:
                                    nc.tensor.matmul(
                                        out=px[:],
                                        lhsT=satrow[:, g * H:(g + 1) * H],
                                        rhs=minvmm[:, t0:t1, di_, :],
                                        start=False, stop=True)
                                nc.vector.tensor_scalar(
                                    out=xp[:, t0:t1, di_, g, :], in0=px[:],
                                    scalar1=bcol[:, g:g + 1], scalar2=None,
                                    op0=ALU.add)

            g3 = g128[:]  # [P, T, BC] view

            # ---- scan machinery
            SB = {d: cpool.tile([P, 9, BC], dt.float32, tag=f"SB{d}",
                                name=f"SB{d}")
                  for d in "fb"}

            def scan_layer(l, y, srcs):
                for di_, d in enumerate("fb"):
                    nc.vector.memset(SB[d][:, 4, :], 0.0)   # C'
                    nc.vector.memset(y[d][:, 0, :], 0.0)    # H'
                wh = {d: wsb[f"wh{l}{d}"] for d in "fb"}
                with tc.tile_pool(name=f"psx{l}", bufs=3, space="PSUM") as psx:
                    # emit xp chunk n just before the scan steps it feeds, so
                    # early scan steps are not queued behind later evacs
                    for tj in range(T):
                        if tj % TCH == 0:
                            xp_chunk_n(l, srcs, psx, tj // TCH)
                        tj1 = tj + 1
                        zps = {}
                        for di_, d in enumerate("fb"):
                            zp = psz.tile([P, 4, BC], dt.float32, tag="zp")
                            nc.tensor.matmul(
                                out=zp[:], lhsT=identh[:],
                                rhs=xp[:, tj, di_, :, :],
                                start=True, stop=False, skip_group_check=True)
                            for g in range(4):
                                nc.tensor.matmul(
                                    out=zp[:, g, :],
                                    lhsT=wh[d][:, g * H:(g + 1) * H],
                                    rhs=y[d][:, tj, :],
                                    start=False, stop=True,
                                    skip_group_check=True)
                            zps[d] = zp
                        for d in "fb":
                            nc.scalar.activation(
                                out=SB[d][:, 0:4, :], in_=zps[d][:],
                                func=AF.Tanh, scale=0.5)
                        for d in "fb":
                            S = SB[d]
                            nc.vector.scalar_tensor_tensor(
                                out=S[:, 5:7, :], in0=S[:, 0:2, :], scalar=1.0,
                                in1=S[:, 2:5:2, :], op0=ALU.add, op1=ALU.mult)
                        for d in "fb":
                            S = SB[d]
                            nc.vector.scalar_tensor_tensor(
                                out=S[:, 4, :], in0=S[:, 6, :], scalar=0.5,
                                in1=S[:, 5, :], op0=ALU.mult, op1=ALU.add)
                        for d in "fb":
                            S = SB[d]
                            nc.scalar.activation(
                                out=S[:, 7, :], in_=S[:, 4, :], func=AF.Tanh,
                                scale=0.5)
                        for di_, d in enumerate("fb"):
                            S = SB[d]
                            nc.vector.scalar_tensor_tensor(
                                out=y[d][:, tj1, :], in0=S[:, 3, :],
                                scalar=1.0, in1=S[:, 7, :], op0=ALU.add,
                                op1=ALU.mult)
                            nc.vector.copy_predicated(
                                out=y[d][:, tj1, :],
                                mask=minv[:, tj, di_, :],
                                data=y[d][:, tj, :])

            scan_layer(0, y0, {
                "f": [("wx0f", g3)],
                "b": [("wx0b", g3[:, ::-1, :])],
            })
            if DBG:
                nc.sync.dma_start(out=dbg["dy0f"][:], in_=y0["f"][:])
                nc.sync.dma_start(out=dbg["dy0b"][:], in_=y0["b"][:])

            # ---- layer-1 xp from SBUF y0
            yf = y0["f"][:, 1:T + 1, :]
            yb = y0["b"][:, 1:T + 1, :]
            yfr = yf[:, ::-1, :]
            ybr = yb[:, ::-1, :]
            scan_layer(1, y1, {
                "f": [("wx1ff", yf), ("wx1fb", ybr)],
                "b": [("wx1bf", yfr), ("wx1bb", yb)],
            })
            if DBG:
                nc.sync.dma_start(out=dbg["dy1f"][:], in_=y1["f"][:])
                nc.sync.dma_start(out=dbg["dy1b"][:], in_=y1["b"][:])

            # ---- pooling over the 256 concat features per token
            fmx = cpool.tile([P, NCH], dt.float32, tag="fmx")
            fsum = cpool.tile([P, NCH], dt.float32, tag="fsum")
            yb_nat = y1["b"][:, 1:T + 1, :][:, ::-1, :]  # natural time
            with tc.tile_pool(name="psp", bufs=2, space="PSUM") as psp, \
                 tc.tile_pool(name="ystg", bufs=3) as ystgp:
                for c in range(NCH):
                    # stage to fp32 SBUF (the copy also un-reverses the
                    # scan-step-major b direction), transpose fp32, reduce.
                    ystg = ystgp.tile([P, 2, 16, BC], dt.float32, tag="ystg")
                    pt = psp.tile([P, 2, P], dt.float32, tag="pt")
                    srcs = (y1["f"][:, 1 + 16 * c:1 + 16 * (c + 1), :],
                            yb_nat[:, 16 * c:16 * (c + 1), :])
                    for di_ in range(2):
                        nc.gpsimd.tensor_copy(
                            out=ystg[:, di_, :, :], in_=srcs[di_])
                        nc.tensor.transpose(
                            out=pt[:, di_, :],
                            in_=ystg[:, di_, :, :],
                            identity=identf[:])
                    nc.vector.tensor_reduce(
                        out=fmx[:, c:c + 1], in_=pt[:],
                        axis=mybir.AxisListType.XYZW, op=ALU.max)
                    nc.vector.tensor_reduce(
                        out=fsum[:, c:c + 1], in_=pt[:],
                        axis=mybir.AxisListType.XYZW, op=ALU.add)

            if DBG:
                nc.sync.dma_start(out=dbg["dfmx"][:], in_=fmx[:])
                nc.sync.dma_start(out=dbg["dfsum"][:], in_=fsum[:])

            # ---- FC head via small DRAM bounce (transposed feat layout)
            with tc.tile_pool(name="ep", bufs=3) as epool, \
                 tc.tile_pool(name="psf", bufs=1, space="PSUM") as psf:
                featv = feat_dram  # [2, T, BC]
                nc.sync.dma_start(
                    out=featv[0].rearrange("(c q) b -> (q b) c", q=16),
                    in_=fmx[:])
                nc.sync.dma_start(
                    out=featv[1].rearrange("(c q) b -> (q b) c", q=16),
                    in_=fsum[:])
                pfc = psf.tile([BC, NCLS], dt.float32, tag="pfc")
                NQ = 2 * T // P
                lqa = epool.tile([P, NQ, BC], dt.float32, tag="lqa")
                nc.sync.dma_start(
                    out=lqa[:],
                    in_=feat_dram.rearrange("s (q p) b -> p (s q) b", p=P))
                for q in range(NQ):
                    nc.tensor.matmul(
                        out=pfc[:], lhsT=lqa[:, q, :], rhs=fcw_t[:, q, :],
                        start=(q == 0), stop=(q == NQ - 1))
                ob = epool.tile([BC, NCLS], dt.float32, tag="ob")
                nc.vector.tensor_tensor(
                    out=ob[:], in0=pfc[:], in1=fcb_t[:], op=ALU.add)
                nc.vector.tensor_scalar(
                    out=ob[:], in0=ob[:], scalar1=0.0, scalar2=None,
                    op0=ALU.max)
                nc.sync.dma_start(out=out_d[:], in_=ob[:])

    split_multi_waits(nc)
    return nc


_cached_nc = None


def _get_nc():
    global _cached_nc
    if _cached_nc is None:
        _install_hook()
        _cached_nc = _build()
    return _cached_nc


def _in_maps(inputs):
    w = _fold_weights(inputs)
    x = np.asarray(inputs["x"]).astype(np.int32)  # [64, 512]
    shared = {
        "emb": w["emb"], "identf": w["identf"], "identh": w["identh"],
        "satrow": w["satrow"], "fcw": w["fcw"], "fcb_rep": w["fcb_rep"],
    }
    for l in (0, 1):
        for d in ("f", "b"):
            shared[f"wh{l}{d}"] = w[f"wh{l}{d}"]
            shared[f"bcol{l}{d}"] = w[f"bcol{l}{d}"]
            if l == 0:
                shared[f"wx0{d}"] = w[f"wx0{d}"]
            else:
                shared[f"wx1{d}f"] = w[f"wx1{d}f"]
                shared[f"wx1{d}b"] = w[f"wx1{d}b"]
    maps = []
    for c in range(NCORES):
        xc = x[c * BC:(c + 1) * BC]            # [BC, T]
        idx = np.ascontiguousarray(xc.T).reshape(-1).astype(np.int32)
        m = (xc != 0).T                        # [T, BC] bool
        minv_f = (~m).astype(np.uint8)         # forward-time inverted
        minv_b = minv_f[::-1]                  # scan-step s <-> t = T-1-s
        mi = np.stack([minv_f, minv_b], axis=1)  # [T, 2, BC]
        minv = np.broadcast_to(mi[None], (P, T, 2, BC))
        maps.append(dict(
            shared, idx=idx,
            minv=np.ascontiguousarray(minv),
            minvmm=np.ascontiguousarray(mi[None]).astype(np.float16),
        ))
    return maps


def _run(inputs, trace=False):
    from concourse.bass_utils import run_bass_kernel_spmd
    nc = _get_nc()
    maps = _in_maps(inputs)
    res = run_bass_kernel_spmd(nc, maps, list(range(NCORES)), trace=trace)
    out = np.concatenate([res.results[c]["out"] for c in range(NCORES)], axis=0)
    return out.astype(np.float32), res


def kernel(**inputs):
    out, _ = _run(inputs, trace=False)
    return out


def run_traced(inputs):
    out, res = _run(inputs, trace=True)
    return out, res
